# revision 8
# baseline (speedup 1.0000x reference)
"""GCN+NNConv (edge-MLP message passing) Trainium2 Bass kernel, 8-core SPMD.

Sharding: edges sorted by dst, sharded by dst range (3750 nodes/core).
Each 128-node block's edges are padded to a fixed 6 edge-tiles so all 8
cores run one identical program. Aggregation is done with one-hot merge
matmuls accumulating in PSUM per node block (no scatters). x[src] and
y[src] are fetched with per-tile indirect DMA gathers. One AllGather
shares y across cores for the GCN layer.
"""
import numpy as np

import concourse.bass as bass
import concourse.mybir as mybir
import concourse.tile as tile

# problem constants (hardcoded per contract)
N = 30000
E = 150000
IN = 16
H = 32
C = 10
IH = IN * H          # 512
NCORES = 8
NLOC = N // NCORES   # 3750
NPAD = 3840          # 30 blocks of 128
NB = NPAD // 128     # 30
P = 128

_COMPILED = {}


def _split_multi_waits(nc, max_waits=1):
    """This walrus build allows only one inline sync-wait per instruction;
    hoist extras into single-wait NOPs on the same engine just before."""
    for fn in nc.m.functions:
        for blk in fn.blocks:
            insts = list(blk.instructions)
            new_insts = []
            for inst in insts:
                si = inst.sync_info
                if si is not None and si.on_wait is not None and len(si.on_wait) > max_waits:
                    waits = list(si.on_wait)
                    keep = waits[-max_waits:]
                    extra = waits[:-max_waits]
                    for j, w in enumerate(extra):
                        nop = mybir.InstNoOp(
                            name=f"{inst.name}-waitnop{j}",
                            engine=inst.engine,
                            ins=[], outs=[],
                            sync_info=mybir.SyncInfo(on_wait=[w], on_update=[]),
                        )
                        new_insts.append(nop)
                    inst.sync_info = mybir.SyncInfo(on_wait=keep, on_update=si.on_update)
                new_insts.append(inst)
            blk.instructions = new_insts
    return nc


def _build(tpb):
    """Build the SPMD Bass kernel. tpb = tiles per block (edge capacity/block/128)."""
    T = NB * tpb                 # edge tiles per core
    NG = (T + 3) // 4            # groups of 4 tiles
    f32 = mybir.dt.float32
    i32 = mybir.dt.int32
    AF = mybir.ActivationFunctionType
    AL = mybir.AluOpType
    AX = mybir.AxisListType

    nc = bass.Bass(num_devices=NCORES)

    # ---- inputs ----
    x_d = nc.dram_tensor("x", [N, IN], f32, kind="ExternalInput")
    GW = (NG + 2) // 3 * 512
    attr_g = nc.dram_tensor("attr_g", [65, GW], f32, kind="ExternalInput")
    dstf_d = nc.dram_tensor("dstf", [P, T], f32, kind="ExternalInput")
    srcix_d = nc.dram_tensor("srcix", [P, T], i32, kind="ExternalInput")
    ysrcix_d = nc.dram_tensor("ysrcix", [P, T], i32, kind="ExternalInput")
    xT_d = nc.dram_tensor("xT", [IN, NPAD], f32, kind="ExternalInput")
    w1p_d = nc.dram_tensor("w1p", [65, IH], f32, kind="ExternalInput")
    b1p_d = nc.dram_tensor("b1p", [P, 4], f32, kind="ExternalInput")
    W2p_d = nc.dram_tensor("W2p", [P, 4, IH], f32, kind="ExternalInput")
    b2p_d = nc.dram_tensor("b2p", [1, IH], f32, kind="ExternalInput")
    root_d = nc.dram_tensor("rootw", [IN, H], f32, kind="ExternalInput")
    bias1_d = nc.dram_tensor("bias1r", [1, H], f32, kind="ExternalInput")
    Wg_d = nc.dram_tensor("Wg", [H, 16], f32, kind="ExternalInput")
    bg_d = nc.dram_tensor("bg_rep", [P, 16], f32, kind="ExternalInput")

    # ---- outputs ----
    out_d = nc.dram_tensor("out_final", [NPAD, C], f32, kind="ExternalOutput")

    with tile.TileContext(nc) as tc:
        with (
            tc.tile_pool(name="cst", bufs=1) as cst,
            tc.tile_pool(name="big", bufs=1) as big,
            tc.tile_pool(name="wk", bufs=3) as wk,
            tc.tile_pool(name="ht", bufs=2) as htp,
            tc.tile_pool(name="ph", bufs=2, space="PSUM") as ph,
            tc.tile_pool(name="pt", bufs=2, space="PSUM") as pt,
            tc.tile_pool(name="pm", bufs=2, space="PSUM") as pm,
            tc.tile_pool(name="pb", bufs=2, space="PSUM") as pb,
            tc.tile_pool(name="dram", bufs=1, space="DRAM") as dram,
        ):
            # ---- load constants / weights ----
            w1p = cst.tile([65, IH], f32)
            nc.gpsimd.dma_start(out=w1p[:], in_=w1p_d[:])
            b1p = cst.tile([P, 4], f32)
            nc.gpsimd.dma_start(out=b1p[:], in_=b1p_d[:])
            W2p = cst.tile([P, 4, IH], f32)
            nc.gpsimd.dma_start(out=W2p[:], in_=W2p_d[:])
            b2p = cst.tile([1, IH], f32)
            nc.gpsimd.dma_start(out=b2p[:], in_=b2p_d[:])
            rootw = cst.tile([IN, H], f32)
            nc.gpsimd.dma_start(out=rootw[:], in_=root_d[:])
            bias1r = cst.tile([1, H], f32)
            nc.gpsimd.dma_start(out=bias1r[:], in_=bias1_d[:])
            Wg = cst.tile([H, 16], f32)
            nc.gpsimd.dma_start(out=Wg[:], in_=Wg_d[:])
            bg_rep = cst.tile([P, 16], f32)
            nc.gpsimd.dma_start(out=bg_rep[:], in_=bg_d[:])
            ag = cst.tile([65, GW], f32)
            nc.gpsimd.dma_start(out=ag[:], in_=attr_g[:])
            dstf = cst.tile([P, T], f32)
            nc.gpsimd.dma_start(out=dstf[:], in_=dstf_d[:])
            srcix = cst.tile([P, T], i32)
            nc.gpsimd.dma_start(out=srcix[:], in_=srcix_d[:])
            ysrcix = cst.tile([P, T], i32)
            nc.gpsimd.dma_start(out=ysrcix[:], in_=ysrcix_d[:])
            xTt = cst.tile([IN, NPAD], f32)
            nc.gpsimd.dma_start(out=xTt[:], in_=xT_d[:])

            iota_i = cst.tile([P, P], i32)
            nc.gpsimd.iota(iota_i[:], pattern=[[1, P]], base=0, channel_multiplier=0)
            iotaf = cst.tile([P, P], f32)
            nc.vector.tensor_copy(out=iotaf[:], in_=iota_i[:])
            ones_row = cst.tile([1, P], f32)
            nc.vector.memset(ones_row[:], 1.0)
            ident = cst.tile([P, P], f32)
            from concourse.masks import make_identity
            make_identity(nc, ident[:])

            # ---- big per-core buffers ----
            xs = big.tile([P, T, IN], f32)          # gathered x[src]
            yg = big.tile([P, T, 16], f32)          # gathered y_full[src]
            summed = big.tile([P, NB, 33], f32)     # phase-A node sums (+count)
            y_own = big.tile([P, NB, 16], f32)
            dinv_a = big.tile([P, NB], f32)

            # ---- phase A-0: gather x[src] for every edge tile ----
            for t in range(T):
                nc.gpsimd.indirect_dma_start(
                    out=xs[:, t, :], out_offset=None, in_=x_d[:],
                    in_offset=bass.IndirectOffsetOnAxis(ap=srcix[:, t:t + 1], axis=0),
                )

            # ---- phase A: per group h, per tile theta/msg/merge ----
            def h_group(u):
                """Compute relu(w1*a+b1) for 4 tiles of group u -> hT [128k, 4kt, 512e]."""
                hT = htp.tile([P, 4, 512], f32, name="hT", tag="hT")
                bp = (u % 3) * 32
                rhs = ag[bp:bp + 1, (u // 3) * 512:(u // 3) * 512 + 512]
                for kt in range(4):
                    hp = ph.tile([P, 512], f32, name="hp", tag="hp")
                    nc.tensor.matmul(out=hp[:], lhsT=w1p[bp:bp + 1, kt * P:(kt + 1) * P],
                                     rhs=rhs, start=True, stop=True)
                    nc.scalar.activation(out=hT[:, kt, :], in_=hp[:], func=AF.Relu,
                                         bias=b1p[:, kt:kt + 1], scale=1.0)
                return hT

            for b in range(NB):
                mps = pm.tile([P, 33], f32, name="mps", tag="mps")
                for j in range(tpb):
                    t = b * tpb + j
                    r = t % 4
                    if r == 0:
                        hT_cur = h_group(t // 4)
                    # theta for tile t
                    th = pt.tile([P, IH], f32, name="th", tag="th")
                    for kt in range(4):
                        nc.tensor.matmul(out=th[:], lhsT=hT_cur[:, kt, r * P:(r + 1) * P],
                                         rhs=W2p[:, kt, :], start=(kt == 0), stop=False)
                    nc.tensor.matmul(out=th[:], lhsT=ones_row[:], rhs=b2p[:],
                                     start=False, stop=True)
                    # msg = sum_i xs[:,t,i] * theta[:, (o,i)]
                    prod = wk.tile([P, IH], f32, name="prod", tag="prod")
                    nc.vector.tensor_tensor(
                        out=prod[:],
                        in0=th[:].rearrange("p (o i) -> p o i", i=IN),
                        in1=xs[:, t, None, :].broadcast_to([P, H, IN]),
                        op=AL.mult,
                    )
                    msg = wk.tile([P, 33], f32, name="msg", tag="msg")
                    nc.vector.tensor_reduce(
                        out=msg[:, :H], in_=prod[:].rearrange("p (o i) -> p o i", i=IN),
                        axis=AX.X, op=AL.add,
                    )
                    nc.vector.memset(msg[:, H:H + 1], 1.0)
                    # merge into node-block psum
                    sh = wk.tile([P, 1], f32, name="sh", tag="sh")
                    nc.vector.tensor_scalar_sub(out=sh[:], in0=dstf[:, t:t + 1],
                                                scalar1=float(128 * b))
                    S = wk.tile([P, P], f32, name="S", tag="S")
                    nc.vector.tensor_tensor(out=S[:], in0=sh[:].to_broadcast([P, P]),
                                            in1=iotaf[:], op=AL.is_equal)
                    nc.tensor.matmul(out=mps[:], lhsT=S[:], rhs=msg[:],
                                     start=(j == 0), stop=(j == tpb - 1))
                nc.scalar.copy(out=summed[:, b, :], in_=mps[:])

            # ---- phase B: per node-block ----
            for b in range(NB):
                cnt = summed[:, b, H:H + 1]
                c1 = wk.tile([P, 1], f32, name="c1", tag="c1")
                nc.vector.tensor_scalar_max(out=c1[:], in0=cnt, scalar1=1.0)
                rec = wk.tile([P, 1], f32, name="rec", tag="rec")
                nc.vector.reciprocal(out=rec[:], in_=c1[:])
                aggr = wk.tile([P, H], f32, name="aggr", tag="aggr")
                nc.vector.tensor_scalar_mul(out=aggr[:], in0=summed[:, b, :H], scalar1=rec[:])
                xr = pb.tile([P, H], f32, name="xr", tag="pb")
                nc.tensor.matmul(out=xr[:], lhsT=xTt[:, b * P:(b + 1) * P], rhs=rootw[:],
                                 start=True, stop=False)
                nc.tensor.matmul(out=xr[:], lhsT=ones_row[:], rhs=bias1r[:],
                                 start=False, stop=True)
                pre = wk.tile([P, H], f32, name="pre", tag="pre")
                nc.vector.tensor_tensor(out=pre[:], in0=aggr[:], in1=xr[:], op=AL.add)
                h1 = wk.tile([P, H], f32, name="h1", tag="h1")
                nc.scalar.activation(out=h1[:], in_=pre[:], func=AF.Relu)
                tp = pb.tile([H, P], f32, name="tp", tag="pb")
                nc.tensor.transpose(out=tp[:], in_=h1[:], identity=ident[:])
                h1T = wk.tile([H, P], f32, name="h1T", tag="h1T")
                nc.vector.tensor_copy(out=h1T[:], in_=tp[:])
                xw = pb.tile([P, 16], f32, name="xw", tag="pb")
                nc.tensor.matmul(out=xw[:], lhsT=h1T[:], rhs=Wg[:], start=True, stop=True)
                d1 = wk.tile([P, 1], f32, name="d1", tag="d1")
                nc.vector.tensor_scalar_add(out=d1[:], in0=cnt, scalar1=1.0)
                r2 = wk.tile([P, 1], f32, name="r2", tag="r2")
                nc.vector.reciprocal(out=r2[:], in_=d1[:])
                nc.scalar.sqrt(out=dinv_a[:, b:b + 1], in_=r2[:])
                ysb = wk.tile([P, 16], f32, name="ysb", tag="ysb")
                nc.vector.memset(ysb[:], 0.0)
                nc.vector.tensor_scalar_mul(out=ysb[:, :C], in0=xw[:, :C],
                                            scalar1=dinv_a[:, b:b + 1])
                nc.vector.tensor_copy(out=y_own[:, b, :], in_=ysb[:])

            # ---- AllGather y ----
            ag_in = dram.tile([NPAD, 16], f32)
            y_full = dram.tile([NCORES * NPAD, 16], f32, addr_space="Shared")
            # copy local slice into internal dram bounce then collective
            for b in range(NB):
                nc.gpsimd.dma_start(out=ag_in[b * P:(b + 1) * P, :], in_=y_own[:, b, :])
            nc.gpsimd.collective_compute(
                "AllGather",
                AL.bypass,
                replica_groups=[list(range(NCORES))],
                ins=[ag_in[:].opt()],
                outs=[y_full[:].opt()],
            )

            # ---- phase C-0: gather y_full[src] ----
            for t in range(T):
                nc.gpsimd.indirect_dma_start(
                    out=yg[:, t, :], out_offset=None, in_=y_full[:],
                    in_offset=bass.IndirectOffsetOnAxis(ap=ysrcix[:, t:t + 1], axis=0),
                )

            # ---- phase C: merge + output ----
            for b in range(NB):
                aps = pm.tile([P, 33], f32, name="aps", tag="mps")
                for j in range(tpb):
                    t = b * tpb + j
                    sh = wk.tile([P, 1], f32, name="sh2", tag="sh")
                    nc.vector.tensor_scalar_sub(out=sh[:], in0=dstf[:, t:t + 1],
                                                scalar1=float(128 * b))
                    S = wk.tile([P, P], f32, name="S2", tag="S")
                    nc.vector.tensor_tensor(out=S[:], in0=sh[:].to_broadcast([P, P]),
                                            in1=iotaf[:], op=AL.is_equal)
                    nc.tensor.matmul(out=aps[:, :16], lhsT=S[:], rhs=yg[:, t, :],
                                     start=(j == 0), stop=(j == tpb - 1))
                t3 = wk.tile([P, 16], f32, name="t3", tag="t3")
                nc.vector.tensor_tensor(out=t3[:], in0=aps[:, :16], in1=y_own[:, b, :], op=AL.add)
                t4 = wk.tile([P, 16], f32, name="t4", tag="t4")
                nc.vector.tensor_scalar_mul(out=t4[:], in0=t3[:], scalar1=dinv_a[:, b:b + 1])
                t5 = wk.tile([P, 16], f32, name="t5", tag="t5")
                nc.vector.tensor_tensor(out=t5[:], in0=t4[:], in1=bg_rep[:], op=AL.add)
                mx = wk.tile([P, 1], f32, name="mx", tag="mx")
                nc.vector.tensor_reduce(out=mx[:], in_=t5[:, :C], axis=AX.X, op=AL.max)
                sh2 = wk.tile([P, C], f32, name="shl", tag="shl")
                nc.vector.tensor_scalar_sub(out=sh2[:], in0=t5[:, :C], scalar1=mx[:])
                ex = wk.tile([P, C], f32, name="ex", tag="ex")
                se = wk.tile([P, 1], f32, name="se", tag="se")
                nc.scalar.activation(out=ex[:], in_=sh2[:], func=AF.Exp, accum_out=se[:])
                lse = wk.tile([P, 1], f32, name="lse", tag="lse")
                nc.scalar.activation(out=lse[:], in_=se[:], func=AF.Ln)
                ofin = wk.tile([P, C], f32, name="ofin", tag="ofin")
                nc.vector.tensor_scalar_sub(out=ofin[:], in0=sh2[:], scalar1=lse[:])
                nc.gpsimd.dma_start(out=out_d[b * P:(b + 1) * P, :], in_=ofin[:])

    _split_multi_waits(nc)
    return nc, T


class _Runner:
    """Jit-once PJRT executor for the SPMD Bass kernel (mirrors
    concourse.bass2jax.run_bass_via_pjrt, but reusable across calls)."""

    def __init__(self, nc):
        import jax
        import numpy as _np
        from jax.sharding import Mesh, PartitionSpec
        from jax.experimental.shard_map import shard_map
        from concourse.bass2jax import (
            install_neuronx_cc_hook, _bass_exec_p, partition_id_tensor,
        )

        install_neuronx_cc_hook()
        self.jax = jax
        pname = nc.partition_id_tensor.name if nc.partition_id_tensor else None
        in_names, out_names, out_avals, zero_outs = [], [], [], []
        for alloc in nc.m.functions[0].allocations:
            if not isinstance(alloc, mybir.MemoryLocationSet):
                continue
            name = alloc.memorylocations[0].name
            if alloc.kind == "ExternalInput":
                if name != pname:
                    in_names.append(name)
            elif alloc.kind == "ExternalOutput":
                out_names.append(name)
                shape = tuple(alloc.tensor_shape)
                dtype = mybir.dt.np(alloc.dtype)
                out_avals.append(jax.core.ShapedArray(shape, dtype))
                zero_outs.append(_np.zeros(shape, dtype))
        self.in_names, self.out_names = in_names, out_names
        self.out_avals, self.zero_outs = out_avals, zero_outs
        n_params, n_outs = len(in_names), len(out_avals)
        all_in = in_names + out_names + ([pname] if pname else [])

        def _body(*args):
            operands = list(args)
            if pname is not None:
                operands.append(partition_id_tensor())
            return tuple(_bass_exec_p.bind(
                *operands, out_avals=tuple(out_avals), in_names=tuple(all_in),
                out_names=tuple(out_names), lowering_input_output_aliases=(),
                sim_require_finite=True, sim_require_nnan=True, nc=nc,
            ))

        devices = jax.devices()[:NCORES]
        self.mesh = Mesh(np.asarray(devices), ("core",))
        in_specs = (PartitionSpec("core"),) * (n_params + n_outs)
        out_specs = (PartitionSpec("core"),) * len(out_names)
        self._fn = jax.jit(
            shard_map(_body, mesh=self.mesh, in_specs=in_specs,
                      out_specs=out_specs, check_rep=False),
            donate_argnums=tuple(range(n_params, n_params + n_outs)),
            keep_unused=True,
        )

    def run(self, in_maps):
        import numpy as _np
        concat_in = [
            _np.concatenate([_np.asarray(in_maps[c][n]) for c in range(NCORES)], axis=0)
            for n in self.in_names
        ]
        zeros = [
            _np.zeros((NCORES * z.shape[0], *z.shape[1:]), z.dtype)
            for z in self.zero_outs
        ]
        outs = self._fn(*concat_in, *zeros)
        self.jax.block_until_ready(outs)
        res = []
        for c in range(NCORES):
            d = {}
            for i, name in enumerate(self.out_names):
                a = _np.asarray(outs[i])
                d[name] = a.reshape(NCORES, *self.out_avals[i].shape)[c]
            res.append(d)
        return res


def _get_compiled(tpb):
    if tpb not in _COMPILED:
        nc, T = _build(tpb)
        _COMPILED[tpb] = (_Runner(nc), T)
    return _COMPILED[tpb]


def _prep_core(core, src_s, dst_s, attr_s, tpb):
    """Build per-core padded, block-quantized edge arrays."""
    T = NB * tpb
    EC = T * P
    lo, hi = core * NLOC, (core + 1) * NLOC
    i0, i1 = np.searchsorted(dst_s, lo), np.searchsorted(dst_s, hi)
    src_c = src_s[i0:i1]
    dstl_c = (dst_s[i0:i1] - lo).astype(np.int64)
    attr_c = attr_s[i0:i1]

    src_pad = np.zeros(EC, np.int64)
    dstl_pad = np.full(EC, NPAD - 1, np.int64)
    attr_pad = np.zeros(EC, np.float32)
    blk = dstl_c // P
    # counts per block
    cnts = np.bincount(blk, minlength=NB)
    if cnts.max() > tpb * P:
        return None  # caller bumps tpb
    starts = np.searchsorted(blk, np.arange(NB))
    for b in range(NB):
        n_b = cnts[b]
        sl = slice(starts[b], starts[b] + n_b)
        base = b * tpb * P
        src_pad[base:base + n_b] = src_c[sl]
        dstl_pad[base:base + n_b] = dstl_c[sl]
        attr_pad[base:base + n_b] = attr_c[sl]

    owner = src_pad // NLOC
    yrow = owner * NPAD + (src_pad - owner * NLOC)

    srcix = src_pad.reshape(T, P).T.astype(np.int32).copy()
    ysrcix = yrow.reshape(T, P).T.astype(np.int32).copy()
    dstf = dstl_pad.reshape(T, P).T.astype(np.float32).copy()

    # attr groups: group u (4 tiles = 512 edges) -> partition (u%3)*32, cols (u//3)*512
    NG = (T + 3) // 4
    GW = (NG + 2) // 3 * 512
    attr_gr = np.zeros((65, GW), np.float32)
    for u in range(NG):
        seg = attr_pad[u * 512:(u + 1) * 512]
        attr_gr[(u % 3) * 32, (u // 3) * 512:(u // 3) * 512 + len(seg)] = seg
    return {"srcix": srcix, "ysrcix": ysrcix, "dstf": dstf, "attr_g": attr_gr}


def kernel(**inputs):
    x = np.asarray(inputs["x"], np.float32)
    ea = np.asarray(inputs["edge_attr"], np.float32).reshape(-1)
    ei = np.asarray(inputs["edge_index"]).astype(np.int64)
    W1 = np.asarray(inputs["W1"], np.float32)
    b1 = np.asarray(inputs["b1"], np.float32)
    W2 = np.asarray(inputs["W2"], np.float32)
    b2 = np.asarray(inputs["b2"], np.float32)
    rootw = np.asarray(inputs["root"], np.float32)
    bias1 = np.asarray(inputs["bias1"], np.float32)
    Wg = np.asarray(inputs["Wg"], np.float32)
    bg = np.asarray(inputs["bg"], np.float32)

    src, dst = ei[0], ei[1]
    order = np.argsort(dst, kind="stable")
    src_s, dst_s, attr_s = src[order], dst[order], ea[order]

    # choose tiles-per-block capacity
    tpb = 6
    per_core = None
    while True:
        per_core = [_prep_core(c, src_s, dst_s, attr_s, tpb) for c in range(NCORES)]
        if all(p is not None for p in per_core):
            break
        tpb += 1

    runner, T = _get_compiled(tpb)

    # weight packing (shared across cores)
    perm = np.arange(IH).reshape(IN, H).T.reshape(-1)   # c'=(o,i) -> orig i*32+o
    w1p = np.zeros((65, IH), np.float32)
    w1p[[0, 32, 64], :] = W1.reshape(1, IH)
    b1p = b1.reshape(4, P).T.astype(np.float32).copy()          # [128, 4]
    W2p = W2[:, perm].reshape(4, P, IH).transpose(1, 0, 2).astype(np.float32).copy()
    b2p = b2[perm].reshape(1, IH).astype(np.float32)
    Wg16 = np.zeros((H, 16), np.float32)
    Wg16[:, :C] = Wg
    bg16 = np.zeros((P, 16), np.float32)
    bg16[:, :C] = bg

    in_maps = []
    for c in range(NCORES):
        pc = per_core[c]
        xT = np.zeros((IN, NPAD), np.float32)
        xT[:, :NLOC] = x[c * NLOC:(c + 1) * NLOC].T
        in_maps.append({
            "x": x,
            "attr_g": pc["attr_g"],
            "dstf": pc["dstf"],
            "srcix": pc["srcix"],
            "ysrcix": pc["ysrcix"],
            "xT": xT,
            "w1p": w1p,
            "b1p": b1p,
            "W2p": W2p,
            "b2p": b2p,
            "rootw": rootw,
            "bias1r": bias1.reshape(1, H),
            "Wg": Wg16,
            "bg_rep": bg16,
        })

    results = runner.run(in_maps)
    out = np.concatenate(
        [results[c]["out_final"][:NLOC] for c in range(NCORES)], axis=0
    )
    return out.astype(np.float32)


# revision 9
# speedup vs baseline: 1.0494x; 1.0494x over previous
"""GCN+NNConv (edge-MLP message passing) Trainium2 Bass kernel, 8-core SPMD.

Sharding: edges sorted by dst, sharded by dst range (3750 nodes/core).
Each 128-node block's edges are padded to a fixed 6 edge-tiles so all 8
cores run one identical program. Aggregation is done with one-hot merge
matmuls accumulating in PSUM per node block (no scatters). x[src] and
y[src] are fetched with per-tile indirect DMA gathers. One AllGather
shares y across cores for the GCN layer.
"""
import numpy as np

import concourse.bass as bass
import concourse.mybir as mybir
import concourse.tile as tile

# problem constants (hardcoded per contract)
N = 30000
E = 150000
IN = 16
H = 32
C = 10
IH = IN * H          # 512
NCORES = 8
NLOC = N // NCORES   # 3750
NPAD = 3840          # 30 blocks of 128
NB = NPAD // 128     # 30
P = 128

_COMPILED = {}


def _split_multi_waits(nc, max_waits=1):
    """This walrus build allows only one inline sync-wait per instruction;
    hoist extras into single-wait NOPs on the same engine just before."""
    for fn in nc.m.functions:
        for blk in fn.blocks:
            insts = list(blk.instructions)
            new_insts = []
            for inst in insts:
                si = inst.sync_info
                if si is not None and si.on_wait is not None and len(si.on_wait) > max_waits:
                    waits = list(si.on_wait)
                    keep = waits[-max_waits:]
                    extra = waits[:-max_waits]
                    for j, w in enumerate(extra):
                        nop = mybir.InstNoOp(
                            name=f"{inst.name}-waitnop{j}",
                            engine=inst.engine,
                            ins=[], outs=[],
                            sync_info=mybir.SyncInfo(on_wait=[w], on_update=[]),
                        )
                        new_insts.append(nop)
                    inst.sync_info = mybir.SyncInfo(on_wait=keep, on_update=si.on_update)
                new_insts.append(inst)
            blk.instructions = new_insts
    return nc


def _build(tpb):
    """Build the SPMD Bass kernel. tpb = tiles per block (edge capacity/block/128)."""
    T = NB * tpb                 # edge tiles per core
    NG = (T + 3) // 4            # groups of 4 tiles
    f32 = mybir.dt.float32
    bf16 = mybir.dt.bfloat16
    i32 = mybir.dt.int32
    AF = mybir.ActivationFunctionType
    AL = mybir.AluOpType
    AX = mybir.AxisListType

    nc = bass.Bass(num_devices=NCORES)

    # ---- inputs ----
    x_d = nc.dram_tensor("x", [N, IN], f32, kind="ExternalInput")
    GW = (NG + 2) // 3 * 512
    attr_g = nc.dram_tensor("attr_g", [65, GW], f32, kind="ExternalInput")
    dstf_d = nc.dram_tensor("dstf", [P, T], f32, kind="ExternalInput")
    srcix_d = nc.dram_tensor("srcix", [P, T], i32, kind="ExternalInput")
    ysrcix_d = nc.dram_tensor("ysrcix", [P, T], i32, kind="ExternalInput")
    xT_d = nc.dram_tensor("xT", [IN, NPAD], f32, kind="ExternalInput")
    w1p_d = nc.dram_tensor("w1p", [65, IH], f32, kind="ExternalInput")
    b1p_d = nc.dram_tensor("b1p", [P, 4], f32, kind="ExternalInput")
    W2p_d = nc.dram_tensor("W2p", [P, 4, IH], bf16, kind="ExternalInput")
    b2p_d = nc.dram_tensor("b2p", [1, IH], bf16, kind="ExternalInput")
    root_d = nc.dram_tensor("rootw", [IN, H], f32, kind="ExternalInput")
    bias1_d = nc.dram_tensor("bias1r", [1, H], f32, kind="ExternalInput")
    Wg_d = nc.dram_tensor("Wg", [H, 16], f32, kind="ExternalInput")
    bg_d = nc.dram_tensor("bg_rep", [P, 16], f32, kind="ExternalInput")

    # ---- outputs ----
    out_d = nc.dram_tensor("out_final", [NPAD, C], f32, kind="ExternalOutput")

    with tile.TileContext(nc) as tc:
        with (
            tc.tile_pool(name="cst", bufs=1) as cst,
            tc.tile_pool(name="big", bufs=1) as big,
            tc.tile_pool(name="wk", bufs=3) as wk,
            tc.tile_pool(name="ht", bufs=2) as htp,
            tc.tile_pool(name="ph", bufs=2, space="PSUM") as ph,
            tc.tile_pool(name="pt", bufs=2, space="PSUM") as pt,
            tc.tile_pool(name="pm", bufs=2, space="PSUM") as pm,
            tc.tile_pool(name="pb", bufs=2, space="PSUM") as pb,
            tc.tile_pool(name="dram", bufs=1, space="DRAM") as dram,
        ):
            # ---- load constants / weights ----
            w1p = cst.tile([65, IH], f32)
            nc.gpsimd.dma_start(out=w1p[:], in_=w1p_d[:])
            b1p = cst.tile([P, 4], f32)
            nc.gpsimd.dma_start(out=b1p[:], in_=b1p_d[:])
            W2p = cst.tile([P, 4, IH], bf16)
            nc.gpsimd.dma_start(out=W2p[:], in_=W2p_d[:])
            b2p = cst.tile([1, IH], bf16)
            nc.gpsimd.dma_start(out=b2p[:], in_=b2p_d[:])
            rootw = cst.tile([IN, H], f32)
            nc.gpsimd.dma_start(out=rootw[:], in_=root_d[:])
            bias1r = cst.tile([1, H], f32)
            nc.gpsimd.dma_start(out=bias1r[:], in_=bias1_d[:])
            Wg = cst.tile([H, 16], f32)
            nc.gpsimd.dma_start(out=Wg[:], in_=Wg_d[:])
            bg_rep = cst.tile([P, 16], f32)
            nc.gpsimd.dma_start(out=bg_rep[:], in_=bg_d[:])
            ag = cst.tile([65, GW], f32)
            nc.gpsimd.dma_start(out=ag[:], in_=attr_g[:])
            dstf = cst.tile([P, T], f32)
            nc.gpsimd.dma_start(out=dstf[:], in_=dstf_d[:])
            srcix = cst.tile([P, T], i32)
            nc.gpsimd.dma_start(out=srcix[:], in_=srcix_d[:])
            ysrcix = cst.tile([P, T], i32)
            nc.gpsimd.dma_start(out=ysrcix[:], in_=ysrcix_d[:])
            xTt = cst.tile([IN, NPAD], f32)
            nc.gpsimd.dma_start(out=xTt[:], in_=xT_d[:])

            iota_i = cst.tile([P, P], i32)
            nc.gpsimd.iota(iota_i[:], pattern=[[1, P]], base=0, channel_multiplier=0)
            iotaf = cst.tile([P, P], f32)
            nc.vector.tensor_copy(out=iotaf[:], in_=iota_i[:])
            ones_row = cst.tile([1, P], f32)
            ones_bf = cst.tile([1, P], bf16)
            nc.vector.memset(ones_bf[:], 1.0)
            nc.vector.memset(ones_row[:], 1.0)
            ident = cst.tile([P, P], f32)
            from concourse.masks import make_identity
            make_identity(nc, ident[:])

            # ---- big per-core buffers ----
            xs = big.tile([P, T, IN], f32)          # gathered x[src]
            yg = big.tile([P, T, 16], f32)          # gathered y_full[src]
            summed = big.tile([P, NB, 33], f32)     # phase-A node sums (+count)
            y_own = big.tile([P, NB, 16], f32)
            dinv_a = big.tile([P, NB], f32)

            # ---- phase A-0: gather x[src] for every edge tile ----
            for t in range(T):
                nc.gpsimd.indirect_dma_start(
                    out=xs[:, t, :], out_offset=None, in_=x_d[:],
                    in_offset=bass.IndirectOffsetOnAxis(ap=srcix[:, t:t + 1], axis=0),
                )

            # ---- phase A: per group h, per tile theta/msg/merge ----
            def h_group(u):
                """Compute relu(w1*a+b1) for 4 tiles of group u -> hT [128k, 4kt, 512e]."""
                hT = htp.tile([P, 4, 512], bf16, name="hT", tag="hT")
                bp = (u % 3) * 32
                rhs = ag[bp:bp + 1, (u // 3) * 512:(u // 3) * 512 + 512]
                for kt in range(4):
                    hp = ph.tile([P, 512], f32, name="hp", tag="hp")
                    nc.tensor.matmul(out=hp[:], lhsT=w1p[bp:bp + 1, kt * P:(kt + 1) * P],
                                     rhs=rhs, start=True, stop=True)
                    nc.scalar.activation(out=hT[:, kt, :], in_=hp[:], func=AF.Relu,
                                         bias=b1p[:, kt:kt + 1], scale=1.0)
                return hT

            for b in range(NB):
                mps = pm.tile([P, 33], f32, name="mps", tag="mps")
                for j in range(tpb):
                    t = b * tpb + j
                    r = t % 4
                    if r == 0:
                        hT_cur = h_group(t // 4)
                    # theta for tile t
                    th = pt.tile([P, IH], f32, name="th", tag="th")
                    for kt in range(4):
                        nc.tensor.matmul(out=th[:], lhsT=hT_cur[:, kt, r * P:(r + 1) * P],
                                         rhs=W2p[:, kt, :], start=(kt == 0), stop=False)
                    nc.tensor.matmul(out=th[:], lhsT=ones_bf[:], rhs=b2p[:],
                                     start=False, stop=True)
                    # msg = sum_i xs[:,t,i] * theta[:, (o,i)]
                    prod = wk.tile([P, IH], f32, name="prod", tag="prod")
                    nc.vector.tensor_tensor(
                        out=prod[:],
                        in0=th[:].rearrange("p (o i) -> p o i", i=IN),
                        in1=xs[:, t, None, :].broadcast_to([P, H, IN]),
                        op=AL.mult,
                    )
                    msg = wk.tile([P, 33], f32, name="msg", tag="msg")
                    nc.vector.tensor_reduce(
                        out=msg[:, :H], in_=prod[:].rearrange("p (o i) -> p o i", i=IN),
                        axis=AX.X, op=AL.add,
                    )
                    nc.vector.memset(msg[:, H:H + 1], 1.0)
                    # merge into node-block psum
                    sh = wk.tile([P, 1], f32, name="sh", tag="sh")
                    nc.vector.tensor_scalar_sub(out=sh[:], in0=dstf[:, t:t + 1],
                                                scalar1=float(128 * b))
                    S = wk.tile([P, P], f32, name="S", tag="S")
                    nc.vector.tensor_tensor(out=S[:], in0=sh[:].to_broadcast([P, P]),
                                            in1=iotaf[:], op=AL.is_equal)
                    nc.tensor.matmul(out=mps[:], lhsT=S[:], rhs=msg[:],
                                     start=(j == 0), stop=(j == tpb - 1))
                nc.scalar.copy(out=summed[:, b, :], in_=mps[:])

            # ---- phase B: per node-block ----
            for b in range(NB):
                cnt = summed[:, b, H:H + 1]
                c1 = wk.tile([P, 1], f32, name="c1", tag="c1")
                nc.vector.tensor_scalar_max(out=c1[:], in0=cnt, scalar1=1.0)
                rec = wk.tile([P, 1], f32, name="rec", tag="rec")
                nc.vector.reciprocal(out=rec[:], in_=c1[:])
                aggr = wk.tile([P, H], f32, name="aggr", tag="aggr")
                nc.vector.tensor_scalar_mul(out=aggr[:], in0=summed[:, b, :H], scalar1=rec[:])
                xr = pb.tile([P, H], f32, name="xr", tag="pb")
                nc.tensor.matmul(out=xr[:], lhsT=xTt[:, b * P:(b + 1) * P], rhs=rootw[:],
                                 start=True, stop=False)
                nc.tensor.matmul(out=xr[:], lhsT=ones_row[:], rhs=bias1r[:],
                                 start=False, stop=True)
                pre = wk.tile([P, H], f32, name="pre", tag="pre")
                nc.vector.tensor_tensor(out=pre[:], in0=aggr[:], in1=xr[:], op=AL.add)
                h1 = wk.tile([P, H], f32, name="h1", tag="h1")
                nc.scalar.activation(out=h1[:], in_=pre[:], func=AF.Relu)
                tp = pb.tile([H, P], f32, name="tp", tag="pb")
                nc.tensor.transpose(out=tp[:], in_=h1[:], identity=ident[:])
                h1T = wk.tile([H, P], f32, name="h1T", tag="h1T")
                nc.vector.tensor_copy(out=h1T[:], in_=tp[:])
                xw = pb.tile([P, 16], f32, name="xw", tag="pb")
                nc.tensor.matmul(out=xw[:], lhsT=h1T[:], rhs=Wg[:], start=True, stop=True)
                d1 = wk.tile([P, 1], f32, name="d1", tag="d1")
                nc.vector.tensor_scalar_add(out=d1[:], in0=cnt, scalar1=1.0)
                r2 = wk.tile([P, 1], f32, name="r2", tag="r2")
                nc.vector.reciprocal(out=r2[:], in_=d1[:])
                nc.scalar.sqrt(out=dinv_a[:, b:b + 1], in_=r2[:])
                ysb = wk.tile([P, 16], f32, name="ysb", tag="ysb")
                nc.vector.memset(ysb[:], 0.0)
                nc.vector.tensor_scalar_mul(out=ysb[:, :C], in0=xw[:, :C],
                                            scalar1=dinv_a[:, b:b + 1])
                nc.vector.tensor_copy(out=y_own[:, b, :], in_=ysb[:])

            # ---- AllGather y ----
            ag_in = dram.tile([NPAD, 16], f32)
            y_full = dram.tile([NCORES * NPAD, 16], f32, addr_space="Shared")
            # copy local slice into internal dram bounce then collective
            for b in range(NB):
                nc.gpsimd.dma_start(out=ag_in[b * P:(b + 1) * P, :], in_=y_own[:, b, :])
            nc.gpsimd.collective_compute(
                "AllGather",
                AL.bypass,
                replica_groups=[list(range(NCORES))],
                ins=[ag_in[:].opt()],
                outs=[y_full[:].opt()],
            )

            # ---- phase C-0: gather y_full[src] ----
            for t in range(T):
                nc.gpsimd.indirect_dma_start(
                    out=yg[:, t, :], out_offset=None, in_=y_full[:],
                    in_offset=bass.IndirectOffsetOnAxis(ap=ysrcix[:, t:t + 1], axis=0),
                )

            # ---- phase C: merge + output ----
            for b in range(NB):
                aps = pm.tile([P, 33], f32, name="aps", tag="mps")
                for j in range(tpb):
                    t = b * tpb + j
                    sh = wk.tile([P, 1], f32, name="sh2", tag="sh")
                    nc.vector.tensor_scalar_sub(out=sh[:], in0=dstf[:, t:t + 1],
                                                scalar1=float(128 * b))
                    S = wk.tile([P, P], f32, name="S2", tag="S")
                    nc.vector.tensor_tensor(out=S[:], in0=sh[:].to_broadcast([P, P]),
                                            in1=iotaf[:], op=AL.is_equal)
                    nc.tensor.matmul(out=aps[:, :16], lhsT=S[:], rhs=yg[:, t, :],
                                     start=(j == 0), stop=(j == tpb - 1))
                t3 = wk.tile([P, 16], f32, name="t3", tag="t3")
                nc.vector.tensor_tensor(out=t3[:], in0=aps[:, :16], in1=y_own[:, b, :], op=AL.add)
                t4 = wk.tile([P, 16], f32, name="t4", tag="t4")
                nc.vector.tensor_scalar_mul(out=t4[:], in0=t3[:], scalar1=dinv_a[:, b:b + 1])
                t5 = wk.tile([P, 16], f32, name="t5", tag="t5")
                nc.vector.tensor_tensor(out=t5[:], in0=t4[:], in1=bg_rep[:], op=AL.add)
                mx = wk.tile([P, 1], f32, name="mx", tag="mx")
                nc.vector.tensor_reduce(out=mx[:], in_=t5[:, :C], axis=AX.X, op=AL.max)
                sh2 = wk.tile([P, C], f32, name="shl", tag="shl")
                nc.vector.tensor_scalar_sub(out=sh2[:], in0=t5[:, :C], scalar1=mx[:])
                ex = wk.tile([P, C], f32, name="ex", tag="ex")
                se = wk.tile([P, 1], f32, name="se", tag="se")
                nc.scalar.activation(out=ex[:], in_=sh2[:], func=AF.Exp, accum_out=se[:])
                lse = wk.tile([P, 1], f32, name="lse", tag="lse")
                nc.scalar.activation(out=lse[:], in_=se[:], func=AF.Ln)
                ofin = wk.tile([P, C], f32, name="ofin", tag="ofin")
                nc.vector.tensor_scalar_sub(out=ofin[:], in0=sh2[:], scalar1=lse[:])
                nc.gpsimd.dma_start(out=out_d[b * P:(b + 1) * P, :], in_=ofin[:])

    _split_multi_waits(nc)
    return nc, T


class _Runner:
    """Jit-once PJRT executor for the SPMD Bass kernel (mirrors
    concourse.bass2jax.run_bass_via_pjrt, but reusable across calls)."""

    def __init__(self, nc):
        import jax
        import numpy as _np
        from jax.sharding import Mesh, PartitionSpec
        from jax.experimental.shard_map import shard_map
        from concourse.bass2jax import (
            install_neuronx_cc_hook, _bass_exec_p, partition_id_tensor,
        )

        install_neuronx_cc_hook()
        self.jax = jax
        pname = nc.partition_id_tensor.name if nc.partition_id_tensor else None
        in_names, out_names, out_avals, zero_outs = [], [], [], []
        for alloc in nc.m.functions[0].allocations:
            if not isinstance(alloc, mybir.MemoryLocationSet):
                continue
            name = alloc.memorylocations[0].name
            if alloc.kind == "ExternalInput":
                if name != pname:
                    in_names.append(name)
            elif alloc.kind == "ExternalOutput":
                out_names.append(name)
                shape = tuple(alloc.tensor_shape)
                dtype = mybir.dt.np(alloc.dtype)
                out_avals.append(jax.core.ShapedArray(shape, dtype))
                zero_outs.append(_np.zeros(shape, dtype))
        self.in_names, self.out_names = in_names, out_names
        self.out_avals, self.zero_outs = out_avals, zero_outs
        n_params, n_outs = len(in_names), len(out_avals)
        all_in = in_names + out_names + ([pname] if pname else [])

        def _body(*args):
            operands = list(args)
            if pname is not None:
                operands.append(partition_id_tensor())
            return tuple(_bass_exec_p.bind(
                *operands, out_avals=tuple(out_avals), in_names=tuple(all_in),
                out_names=tuple(out_names), lowering_input_output_aliases=(),
                sim_require_finite=True, sim_require_nnan=True, nc=nc,
            ))

        devices = jax.devices()[:NCORES]
        self.mesh = Mesh(np.asarray(devices), ("core",))
        in_specs = (PartitionSpec("core"),) * (n_params + n_outs)
        out_specs = (PartitionSpec("core"),) * len(out_names)
        self._fn = jax.jit(
            shard_map(_body, mesh=self.mesh, in_specs=in_specs,
                      out_specs=out_specs, check_rep=False),
            donate_argnums=tuple(range(n_params, n_params + n_outs)),
            keep_unused=True,
        )

    def run(self, in_maps):
        import numpy as _np
        concat_in = [
            _np.concatenate([_np.asarray(in_maps[c][n]) for c in range(NCORES)], axis=0)
            for n in self.in_names
        ]
        zeros = [
            _np.zeros((NCORES * z.shape[0], *z.shape[1:]), z.dtype)
            for z in self.zero_outs
        ]
        outs = self._fn(*concat_in, *zeros)
        self.jax.block_until_ready(outs)
        res = []
        for c in range(NCORES):
            d = {}
            for i, name in enumerate(self.out_names):
                a = _np.asarray(outs[i])
                d[name] = a.reshape(NCORES, *self.out_avals[i].shape)[c]
            res.append(d)
        return res


def _get_compiled(tpb):
    if tpb not in _COMPILED:
        nc, T = _build(tpb)
        _COMPILED[tpb] = (_Runner(nc), T)
    return _COMPILED[tpb]


def _prep_core(core, src_s, dst_s, attr_s, tpb):
    """Build per-core padded, block-quantized edge arrays."""
    T = NB * tpb
    EC = T * P
    lo, hi = core * NLOC, (core + 1) * NLOC
    i0, i1 = np.searchsorted(dst_s, lo), np.searchsorted(dst_s, hi)
    src_c = src_s[i0:i1]
    dstl_c = (dst_s[i0:i1] - lo).astype(np.int64)
    attr_c = attr_s[i0:i1]

    src_pad = np.zeros(EC, np.int64)
    dstl_pad = np.full(EC, NPAD - 1, np.int64)
    attr_pad = np.zeros(EC, np.float32)
    blk = dstl_c // P
    # counts per block
    cnts = np.bincount(blk, minlength=NB)
    if cnts.max() > tpb * P:
        return None  # caller bumps tpb
    starts = np.searchsorted(blk, np.arange(NB))
    for b in range(NB):
        n_b = cnts[b]
        sl = slice(starts[b], starts[b] + n_b)
        base = b * tpb * P
        src_pad[base:base + n_b] = src_c[sl]
        dstl_pad[base:base + n_b] = dstl_c[sl]
        attr_pad[base:base + n_b] = attr_c[sl]

    owner = src_pad // NLOC
    yrow = owner * NPAD + (src_pad - owner * NLOC)

    srcix = src_pad.reshape(T, P).T.astype(np.int32).copy()
    ysrcix = yrow.reshape(T, P).T.astype(np.int32).copy()
    dstf = dstl_pad.reshape(T, P).T.astype(np.float32).copy()

    # attr groups: group u (4 tiles = 512 edges) -> partition (u%3)*32, cols (u//3)*512
    NG = (T + 3) // 4
    GW = (NG + 2) // 3 * 512
    attr_gr = np.zeros((65, GW), np.float32)
    for u in range(NG):
        seg = attr_pad[u * 512:(u + 1) * 512]
        attr_gr[(u % 3) * 32, (u // 3) * 512:(u // 3) * 512 + len(seg)] = seg
    return {"srcix": srcix, "ysrcix": ysrcix, "dstf": dstf, "attr_g": attr_gr}


def kernel(**inputs):
    x = np.asarray(inputs["x"], np.float32)
    ea = np.asarray(inputs["edge_attr"], np.float32).reshape(-1)
    ei = np.asarray(inputs["edge_index"]).astype(np.int64)
    W1 = np.asarray(inputs["W1"], np.float32)
    b1 = np.asarray(inputs["b1"], np.float32)
    W2 = np.asarray(inputs["W2"], np.float32)
    b2 = np.asarray(inputs["b2"], np.float32)
    rootw = np.asarray(inputs["root"], np.float32)
    bias1 = np.asarray(inputs["bias1"], np.float32)
    Wg = np.asarray(inputs["Wg"], np.float32)
    bg = np.asarray(inputs["bg"], np.float32)

    src, dst = ei[0], ei[1]
    order = np.argsort(dst, kind="stable")
    src_s, dst_s, attr_s = src[order], dst[order], ea[order]

    # choose tiles-per-block capacity
    tpb = 6
    per_core = None
    while True:
        per_core = [_prep_core(c, src_s, dst_s, attr_s, tpb) for c in range(NCORES)]
        if all(p is not None for p in per_core):
            break
        tpb += 1

    runner, T = _get_compiled(tpb)

    # weight packing (shared across cores)
    perm = np.arange(IH).reshape(IN, H).T.reshape(-1)   # c'=(o,i) -> orig i*32+o
    w1p = np.zeros((65, IH), np.float32)
    w1p[[0, 32, 64], :] = W1.reshape(1, IH)
    b1p = b1.reshape(4, P).T.astype(np.float32).copy()          # [128, 4]
    import ml_dtypes
    W2p = W2[:, perm].reshape(4, P, IH).transpose(1, 0, 2).astype(ml_dtypes.bfloat16).copy()
    b2p = b2[perm].reshape(1, IH).astype(ml_dtypes.bfloat16)
    Wg16 = np.zeros((H, 16), np.float32)
    Wg16[:, :C] = Wg
    bg16 = np.zeros((P, 16), np.float32)
    bg16[:, :C] = bg

    in_maps = []
    for c in range(NCORES):
        pc = per_core[c]
        xT = np.zeros((IN, NPAD), np.float32)
        xT[:, :NLOC] = x[c * NLOC:(c + 1) * NLOC].T
        in_maps.append({
            "x": x,
            "attr_g": pc["attr_g"],
            "dstf": pc["dstf"],
            "srcix": pc["srcix"],
            "ysrcix": pc["ysrcix"],
            "xT": xT,
            "w1p": w1p,
            "b1p": b1p,
            "W2p": W2p,
            "b2p": b2p,
            "rootw": rootw,
            "bias1r": bias1.reshape(1, H),
            "Wg": Wg16,
            "bg_rep": bg16,
        })

    results = runner.run(in_maps)
    out = np.concatenate(
        [results[c]["out_final"][:NLOC] for c in range(NCORES)], axis=0
    )
    return out.astype(np.float32)


# revision 10
# speedup vs baseline: 1.1601x; 1.1055x over previous
"""GCN+NNConv (edge-MLP message passing) Trainium2 Bass kernel, 8-core SPMD.

Sharding: edges sorted by dst, sharded by dst range (3750 nodes/core).
Each 128-node block's edges are padded to a fixed 6 edge-tiles so all 8
cores run one identical program. Aggregation is done with one-hot merge
matmuls accumulating in PSUM per node block (no scatters). x[src] and
y[src] are fetched with per-tile indirect DMA gathers. One AllGather
shares y across cores for the GCN layer.
"""
import numpy as np

import concourse.bass as bass
import concourse.mybir as mybir
import concourse.tile as tile

# problem constants (hardcoded per contract)
N = 30000
E = 150000
IN = 16
H = 32
C = 10
IH = IN * H          # 512
NCORES = 8
NLOC = N // NCORES   # 3750
NPAD = 3840          # 30 blocks of 128
NB = NPAD // 128     # 30
P = 128

_COMPILED = {}


def _split_multi_waits(nc, max_waits=1):
    """This walrus build allows only one inline sync-wait per instruction;
    hoist extras into single-wait NOPs on the same engine just before."""
    for fn in nc.m.functions:
        for blk in fn.blocks:
            insts = list(blk.instructions)
            new_insts = []
            for inst in insts:
                si = inst.sync_info
                if si is not None and si.on_wait is not None and len(si.on_wait) > max_waits:
                    waits = list(si.on_wait)
                    keep = waits[-max_waits:]
                    extra = waits[:-max_waits]
                    for j, w in enumerate(extra):
                        nop = mybir.InstNoOp(
                            name=f"{inst.name}-waitnop{j}",
                            engine=inst.engine,
                            ins=[], outs=[],
                            sync_info=mybir.SyncInfo(on_wait=[w], on_update=[]),
                        )
                        new_insts.append(nop)
                    inst.sync_info = mybir.SyncInfo(on_wait=keep, on_update=si.on_update)
                new_insts.append(inst)
            blk.instructions = new_insts
    return nc


def _build(tpb):
    """Build the SPMD Bass kernel. tpb = tiles per block (edge capacity/block/128)."""
    T = NB * tpb                 # edge tiles per core
    NG = (T + 3) // 4            # groups of 4 tiles
    f32 = mybir.dt.float32
    bf16 = mybir.dt.bfloat16
    i32 = mybir.dt.int32
    AF = mybir.ActivationFunctionType
    AL = mybir.AluOpType
    AX = mybir.AxisListType

    nc = bass.Bass(num_devices=NCORES)

    # ---- inputs ----
    x_d = nc.dram_tensor("x", [N, IN], f32, kind="ExternalInput")
    GW = (NG + 2) // 3 * 512
    attr_g = nc.dram_tensor("attr_g", [65, GW], bf16, kind="ExternalInput")
    dstf_d = nc.dram_tensor("dstf", [P, T], f32, kind="ExternalInput")
    srcix_d = nc.dram_tensor("srcix", [P, T], i32, kind="ExternalInput")
    ysrcix_d = nc.dram_tensor("ysrcix", [P, T], i32, kind="ExternalInput")
    xT_d = nc.dram_tensor("xT", [IN, NPAD], f32, kind="ExternalInput")
    w1p_d = nc.dram_tensor("w1p", [65, IH], bf16, kind="ExternalInput")
    b1p_d = nc.dram_tensor("b1p", [P, 4], f32, kind="ExternalInput")
    W2p_d = nc.dram_tensor("W2p", [P, 4, IH], bf16, kind="ExternalInput")
    b2p_d = nc.dram_tensor("b2p", [1, IH], bf16, kind="ExternalInput")
    root_d = nc.dram_tensor("rootw", [IN, H], f32, kind="ExternalInput")
    bias1_d = nc.dram_tensor("bias1r", [1, H], f32, kind="ExternalInput")
    Wg_d = nc.dram_tensor("Wg", [H, 16], f32, kind="ExternalInput")
    bg_d = nc.dram_tensor("bg_rep", [P, 16], f32, kind="ExternalInput")

    # ---- outputs ----
    out_d = nc.dram_tensor("out_final", [NPAD, C], f32, kind="ExternalOutput")

    with tile.TileContext(nc) as tc:
        with (
            tc.tile_pool(name="cst", bufs=1) as cst,
            tc.tile_pool(name="big", bufs=1) as big,
            tc.tile_pool(name="wk", bufs=3) as wk,
            tc.tile_pool(name="ht", bufs=2) as htp,
            tc.tile_pool(name="ph", bufs=2, space="PSUM") as ph,
            tc.tile_pool(name="pt", bufs=2, space="PSUM") as pt,
            tc.tile_pool(name="pm", bufs=2, space="PSUM") as pm,
            tc.tile_pool(name="pb", bufs=2, space="PSUM") as pb,
            tc.tile_pool(name="dram", bufs=1, space="DRAM") as dram,
        ):
            # ---- load constants / weights ----
            w1p = cst.tile([65, IH], bf16)
            nc.gpsimd.dma_start(out=w1p[:], in_=w1p_d[:])
            b1p = cst.tile([P, 4], f32)
            nc.gpsimd.dma_start(out=b1p[:], in_=b1p_d[:])
            W2p = cst.tile([P, 4, IH], bf16)
            nc.gpsimd.dma_start(out=W2p[:], in_=W2p_d[:])
            b2p = cst.tile([1, IH], bf16)
            nc.gpsimd.dma_start(out=b2p[:], in_=b2p_d[:])
            rootw = cst.tile([IN, H], f32)
            nc.gpsimd.dma_start(out=rootw[:], in_=root_d[:])
            bias1r = cst.tile([1, H], f32)
            nc.gpsimd.dma_start(out=bias1r[:], in_=bias1_d[:])
            Wg = cst.tile([H, 16], f32)
            nc.gpsimd.dma_start(out=Wg[:], in_=Wg_d[:])
            bg_rep = cst.tile([P, 16], f32)
            nc.gpsimd.dma_start(out=bg_rep[:], in_=bg_d[:])
            ag = cst.tile([65, GW], bf16)
            nc.gpsimd.dma_start(out=ag[:], in_=attr_g[:])
            dstf = cst.tile([P, T], f32)
            nc.gpsimd.dma_start(out=dstf[:], in_=dstf_d[:])
            srcix = cst.tile([P, T], i32)
            nc.gpsimd.dma_start(out=srcix[:], in_=srcix_d[:])
            ysrcix = cst.tile([P, T], i32)
            nc.gpsimd.dma_start(out=ysrcix[:], in_=ysrcix_d[:])
            xTt = cst.tile([IN, NPAD], f32)
            nc.gpsimd.dma_start(out=xTt[:], in_=xT_d[:])

            iota_i = cst.tile([P, P], i32)
            nc.gpsimd.iota(iota_i[:], pattern=[[1, P]], base=0, channel_multiplier=0)
            iotaf = cst.tile([P, P], f32)
            nc.vector.tensor_copy(out=iotaf[:], in_=iota_i[:])
            ones_row = cst.tile([1, P], f32)
            ones_bf = cst.tile([1, P], bf16)
            nc.vector.memset(ones_bf[:], 1.0)
            nc.vector.memset(ones_row[:], 1.0)
            ident = cst.tile([P, P], f32)
            from concourse.masks import make_identity
            make_identity(nc, ident[:])

            # ---- big per-core buffers ----
            xs = big.tile([P, T, IN], f32)          # gathered x[src]
            yg = big.tile([P, T, 16], f32)          # gathered y_full[src]
            summed = big.tile([P, NB, 33], f32)     # phase-A node sums (+count)
            y_own = big.tile([P, NB, 16], f32)
            dinv_a = big.tile([P, NB], f32)

            # ---- phase A-0: gather x[src] for every edge tile ----
            for t in range(T):
                nc.gpsimd.indirect_dma_start(
                    out=xs[:, t, :], out_offset=None, in_=x_d[:],
                    in_offset=bass.IndirectOffsetOnAxis(ap=srcix[:, t:t + 1], axis=0),
                )

            # ---- phase A: per group h, per tile theta/msg/merge ----
            def h_group(u):
                """Compute relu(w1*a+b1) for 4 tiles of group u -> hT [128k, 4kt, 512e]."""
                hT = htp.tile([P, 4, 512], bf16, name="hT", tag="hT")
                bp = (u % 3) * 32
                rhs = ag[bp:bp + 1, (u // 3) * 512:(u // 3) * 512 + 512]
                for kt in range(4):
                    hp = ph.tile([P, 512], f32, name="hp", tag="hp")
                    nc.tensor.matmul(out=hp[:], lhsT=w1p[bp:bp + 1, kt * P:(kt + 1) * P],
                                     rhs=rhs, start=True, stop=True)
                    nc.scalar.activation(out=hT[:, kt, :], in_=hp[:], func=AF.Relu,
                                         bias=b1p[:, kt:kt + 1], scale=1.0)
                return hT

            for b in range(NB):
                mps = pm.tile([P, 33], f32, name="mps", tag="mps")
                for j in range(tpb):
                    t = b * tpb + j
                    r = t % 4
                    if r == 0:
                        hT_cur = h_group(t // 4)
                    # theta for tile t
                    th = pt.tile([P, IH], f32, name="th", tag="th")
                    for kt in range(4):
                        nc.tensor.matmul(out=th[:], lhsT=hT_cur[:, kt, r * P:(r + 1) * P],
                                         rhs=W2p[:, kt, :], start=(kt == 0), stop=False)
                    nc.tensor.matmul(out=th[:], lhsT=ones_bf[:], rhs=b2p[:],
                                     start=False, stop=True)
                    # msg = sum_i xs[:,t,i] * theta[:, (o,i)]
                    prod = wk.tile([P, IH], f32, name="prod", tag="prod")
                    nc.vector.tensor_tensor(
                        out=prod[:],
                        in0=th[:].rearrange("p (o i) -> p o i", i=IN),
                        in1=xs[:, t, None, :].broadcast_to([P, H, IN]),
                        op=AL.mult,
                    )
                    msg = wk.tile([P, 33], f32, name="msg", tag="msg")
                    nc.vector.tensor_reduce(
                        out=msg[:, :H], in_=prod[:].rearrange("p (o i) -> p o i", i=IN),
                        axis=AX.X, op=AL.add,
                    )
                    nc.vector.memset(msg[:, H:H + 1], 1.0)
                    # merge into node-block psum
                    sh = wk.tile([P, 1], f32, name="sh", tag="sh")
                    nc.vector.tensor_scalar_sub(out=sh[:], in0=dstf[:, t:t + 1],
                                                scalar1=float(128 * b))
                    S = wk.tile([P, P], f32, name="S", tag="S")
                    nc.vector.tensor_tensor(out=S[:], in0=sh[:].to_broadcast([P, P]),
                                            in1=iotaf[:], op=AL.is_equal)
                    nc.tensor.matmul(out=mps[:], lhsT=S[:], rhs=msg[:],
                                     start=(j == 0), stop=(j == tpb - 1))
                nc.scalar.copy(out=summed[:, b, :], in_=mps[:])

            # ---- phase B: per node-block ----
            for b in range(NB):
                cnt = summed[:, b, H:H + 1]
                c1 = wk.tile([P, 1], f32, name="c1", tag="c1")
                nc.vector.tensor_scalar_max(out=c1[:], in0=cnt, scalar1=1.0)
                rec = wk.tile([P, 1], f32, name="rec", tag="rec")
                nc.vector.reciprocal(out=rec[:], in_=c1[:])
                aggr = wk.tile([P, H], f32, name="aggr", tag="aggr")
                nc.vector.tensor_scalar_mul(out=aggr[:], in0=summed[:, b, :H], scalar1=rec[:])
                xr = pb.tile([P, H], f32, name="xr", tag="pb")
                nc.tensor.matmul(out=xr[:], lhsT=xTt[:, b * P:(b + 1) * P], rhs=rootw[:],
                                 start=True, stop=False)
                nc.tensor.matmul(out=xr[:], lhsT=ones_row[:], rhs=bias1r[:],
                                 start=False, stop=True)
                pre = wk.tile([P, H], f32, name="pre", tag="pre")
                nc.vector.tensor_tensor(out=pre[:], in0=aggr[:], in1=xr[:], op=AL.add)
                h1 = wk.tile([P, H], f32, name="h1", tag="h1")
                nc.scalar.activation(out=h1[:], in_=pre[:], func=AF.Relu)
                tp = pb.tile([H, P], f32, name="tp", tag="pb")
                nc.tensor.transpose(out=tp[:], in_=h1[:], identity=ident[:])
                h1T = wk.tile([H, P], f32, name="h1T", tag="h1T")
                nc.vector.tensor_copy(out=h1T[:], in_=tp[:])
                xw = pb.tile([P, 16], f32, name="xw", tag="pb")
                nc.tensor.matmul(out=xw[:], lhsT=h1T[:], rhs=Wg[:], start=True, stop=True)
                d1 = wk.tile([P, 1], f32, name="d1", tag="d1")
                nc.vector.tensor_scalar_add(out=d1[:], in0=cnt, scalar1=1.0)
                r2 = wk.tile([P, 1], f32, name="r2", tag="r2")
                nc.vector.reciprocal(out=r2[:], in_=d1[:])
                nc.scalar.sqrt(out=dinv_a[:, b:b + 1], in_=r2[:])
                ysb = wk.tile([P, 16], f32, name="ysb", tag="ysb")
                nc.vector.memset(ysb[:], 0.0)
                nc.vector.tensor_scalar_mul(out=ysb[:, :C], in0=xw[:, :C],
                                            scalar1=dinv_a[:, b:b + 1])
                nc.vector.tensor_copy(out=y_own[:, b, :], in_=ysb[:])

            # ---- AllGather y ----
            ag_in = dram.tile([NPAD, 16], f32)
            y_full = dram.tile([NCORES * NPAD, 16], f32, addr_space="Shared")
            # copy local slice into internal dram bounce then collective
            for b in range(NB):
                nc.gpsimd.dma_start(out=ag_in[b * P:(b + 1) * P, :], in_=y_own[:, b, :])
            nc.gpsimd.collective_compute(
                "AllGather",
                AL.bypass,
                replica_groups=[list(range(NCORES))],
                ins=[ag_in[:].opt()],
                outs=[y_full[:].opt()],
            )

            # ---- phase C-0: gather y_full[src] ----
            for t in range(T):
                nc.gpsimd.indirect_dma_start(
                    out=yg[:, t, :], out_offset=None, in_=y_full[:],
                    in_offset=bass.IndirectOffsetOnAxis(ap=ysrcix[:, t:t + 1], axis=0),
                )

            # ---- phase C: merge + output ----
            for b in range(NB):
                aps = pm.tile([P, 33], f32, name="aps", tag="mps")
                for j in range(tpb):
                    t = b * tpb + j
                    sh = wk.tile([P, 1], f32, name="sh2", tag="sh")
                    nc.vector.tensor_scalar_sub(out=sh[:], in0=dstf[:, t:t + 1],
                                                scalar1=float(128 * b))
                    S = wk.tile([P, P], f32, name="S2", tag="S")
                    nc.vector.tensor_tensor(out=S[:], in0=sh[:].to_broadcast([P, P]),
                                            in1=iotaf[:], op=AL.is_equal)
                    nc.tensor.matmul(out=aps[:, :16], lhsT=S[:], rhs=yg[:, t, :],
                                     start=(j == 0), stop=(j == tpb - 1))
                t3 = wk.tile([P, 16], f32, name="t3", tag="t3")
                nc.vector.tensor_tensor(out=t3[:], in0=aps[:, :16], in1=y_own[:, b, :], op=AL.add)
                t4 = wk.tile([P, 16], f32, name="t4", tag="t4")
                nc.vector.tensor_scalar_mul(out=t4[:], in0=t3[:], scalar1=dinv_a[:, b:b + 1])
                t5 = wk.tile([P, 16], f32, name="t5", tag="t5")
                nc.vector.tensor_tensor(out=t5[:], in0=t4[:], in1=bg_rep[:], op=AL.add)
                mx = wk.tile([P, 1], f32, name="mx", tag="mx")
                nc.vector.tensor_reduce(out=mx[:], in_=t5[:, :C], axis=AX.X, op=AL.max)
                sh2 = wk.tile([P, C], f32, name="shl", tag="shl")
                nc.vector.tensor_scalar_sub(out=sh2[:], in0=t5[:, :C], scalar1=mx[:])
                ex = wk.tile([P, C], f32, name="ex", tag="ex")
                se = wk.tile([P, 1], f32, name="se", tag="se")
                nc.scalar.activation(out=ex[:], in_=sh2[:], func=AF.Exp, accum_out=se[:])
                lse = wk.tile([P, 1], f32, name="lse", tag="lse")
                nc.scalar.activation(out=lse[:], in_=se[:], func=AF.Ln)
                ofin = wk.tile([P, C], f32, name="ofin", tag="ofin")
                nc.vector.tensor_scalar_sub(out=ofin[:], in0=sh2[:], scalar1=lse[:])
                nc.gpsimd.dma_start(out=out_d[b * P:(b + 1) * P, :], in_=ofin[:])

    _split_multi_waits(nc)
    return nc, T


class _Runner:
    """Jit-once PJRT executor for the SPMD Bass kernel (mirrors
    concourse.bass2jax.run_bass_via_pjrt, but reusable across calls)."""

    def __init__(self, nc):
        import jax
        import numpy as _np
        from jax.sharding import Mesh, PartitionSpec
        from jax.experimental.shard_map import shard_map
        from concourse.bass2jax import (
            install_neuronx_cc_hook, _bass_exec_p, partition_id_tensor,
        )

        install_neuronx_cc_hook()
        self.jax = jax
        pname = nc.partition_id_tensor.name if nc.partition_id_tensor else None
        in_names, out_names, out_avals, zero_outs = [], [], [], []
        for alloc in nc.m.functions[0].allocations:
            if not isinstance(alloc, mybir.MemoryLocationSet):
                continue
            name = alloc.memorylocations[0].name
            if alloc.kind == "ExternalInput":
                if name != pname:
                    in_names.append(name)
            elif alloc.kind == "ExternalOutput":
                out_names.append(name)
                shape = tuple(alloc.tensor_shape)
                dtype = mybir.dt.np(alloc.dtype)
                out_avals.append(jax.core.ShapedArray(shape, dtype))
                zero_outs.append(_np.zeros(shape, dtype))
        self.in_names, self.out_names = in_names, out_names
        self.out_avals, self.zero_outs = out_avals, zero_outs
        n_params, n_outs = len(in_names), len(out_avals)
        all_in = in_names + out_names + ([pname] if pname else [])

        def _body(*args):
            operands = list(args)
            if pname is not None:
                operands.append(partition_id_tensor())
            return tuple(_bass_exec_p.bind(
                *operands, out_avals=tuple(out_avals), in_names=tuple(all_in),
                out_names=tuple(out_names), lowering_input_output_aliases=(),
                sim_require_finite=True, sim_require_nnan=True, nc=nc,
            ))

        devices = jax.devices()[:NCORES]
        self.mesh = Mesh(np.asarray(devices), ("core",))
        in_specs = (PartitionSpec("core"),) * (n_params + n_outs)
        out_specs = (PartitionSpec("core"),) * len(out_names)
        self._fn = jax.jit(
            shard_map(_body, mesh=self.mesh, in_specs=in_specs,
                      out_specs=out_specs, check_rep=False),
            donate_argnums=tuple(range(n_params, n_params + n_outs)),
            keep_unused=True,
        )

    def run(self, in_maps):
        import numpy as _np
        concat_in = [
            _np.concatenate([_np.asarray(in_maps[c][n]) for c in range(NCORES)], axis=0)
            for n in self.in_names
        ]
        zeros = [
            _np.zeros((NCORES * z.shape[0], *z.shape[1:]), z.dtype)
            for z in self.zero_outs
        ]
        outs = self._fn(*concat_in, *zeros)
        self.jax.block_until_ready(outs)
        res = []
        for c in range(NCORES):
            d = {}
            for i, name in enumerate(self.out_names):
                a = _np.asarray(outs[i])
                d[name] = a.reshape(NCORES, *self.out_avals[i].shape)[c]
            res.append(d)
        return res


def _get_compiled(tpb):
    if tpb not in _COMPILED:
        nc, T = _build(tpb)
        _COMPILED[tpb] = (_Runner(nc), T)
    return _COMPILED[tpb]


def _prep_core(core, src_s, dst_s, attr_s, tpb):
    """Build per-core padded, block-quantized edge arrays."""
    T = NB * tpb
    EC = T * P
    lo, hi = core * NLOC, (core + 1) * NLOC
    i0, i1 = np.searchsorted(dst_s, lo), np.searchsorted(dst_s, hi)
    src_c = src_s[i0:i1]
    dstl_c = (dst_s[i0:i1] - lo).astype(np.int64)
    attr_c = attr_s[i0:i1]

    src_pad = np.zeros(EC, np.int64)
    dstl_pad = np.full(EC, NPAD - 1, np.int64)
    attr_pad = np.zeros(EC, np.float32)
    blk = dstl_c // P
    # counts per block
    cnts = np.bincount(blk, minlength=NB)
    if cnts.max() > tpb * P:
        return None  # caller bumps tpb
    starts = np.searchsorted(blk, np.arange(NB))
    for b in range(NB):
        n_b = cnts[b]
        sl = slice(starts[b], starts[b] + n_b)
        base = b * tpb * P
        src_pad[base:base + n_b] = src_c[sl]
        dstl_pad[base:base + n_b] = dstl_c[sl]
        attr_pad[base:base + n_b] = attr_c[sl]

    owner = src_pad // NLOC
    yrow = owner * NPAD + (src_pad - owner * NLOC)

    srcix = src_pad.reshape(T, P).T.astype(np.int32).copy()
    ysrcix = yrow.reshape(T, P).T.astype(np.int32).copy()
    dstf = dstl_pad.reshape(T, P).T.astype(np.float32).copy()

    # attr groups: group u (4 tiles = 512 edges) -> partition (u%3)*32, cols (u//3)*512
    NG = (T + 3) // 4
    GW = (NG + 2) // 3 * 512
    import ml_dtypes as _mld
    attr_gr = np.zeros((65, GW), _mld.bfloat16)
    for u in range(NG):
        seg = attr_pad[u * 512:(u + 1) * 512]
        attr_gr[(u % 3) * 32, (u // 3) * 512:(u // 3) * 512 + len(seg)] = seg
    return {"srcix": srcix, "ysrcix": ysrcix, "dstf": dstf, "attr_g": attr_gr}


def kernel(**inputs):
    x = np.asarray(inputs["x"], np.float32)
    ea = np.asarray(inputs["edge_attr"], np.float32).reshape(-1)
    ei = np.asarray(inputs["edge_index"]).astype(np.int64)
    W1 = np.asarray(inputs["W1"], np.float32)
    b1 = np.asarray(inputs["b1"], np.float32)
    W2 = np.asarray(inputs["W2"], np.float32)
    b2 = np.asarray(inputs["b2"], np.float32)
    rootw = np.asarray(inputs["root"], np.float32)
    bias1 = np.asarray(inputs["bias1"], np.float32)
    Wg = np.asarray(inputs["Wg"], np.float32)
    bg = np.asarray(inputs["bg"], np.float32)

    src, dst = ei[0], ei[1]
    order = np.argsort(dst, kind="stable")
    src_s, dst_s, attr_s = src[order], dst[order], ea[order]

    # choose tiles-per-block capacity
    tpb = 6
    per_core = None
    while True:
        per_core = [_prep_core(c, src_s, dst_s, attr_s, tpb) for c in range(NCORES)]
        if all(p is not None for p in per_core):
            break
        tpb += 1

    runner, T = _get_compiled(tpb)

    # weight packing (shared across cores)
    perm = np.arange(IH).reshape(IN, H).T.reshape(-1)   # c'=(o,i) -> orig i*32+o
    import ml_dtypes
    w1p = np.zeros((65, IH), ml_dtypes.bfloat16)
    w1p[[0, 32, 64], :] = W1.reshape(1, IH).astype(ml_dtypes.bfloat16)
    b1p = b1.reshape(4, P).T.astype(np.float32).copy()          # [128, 4]
    import ml_dtypes
    W2p = W2[:, perm].reshape(4, P, IH).transpose(1, 0, 2).astype(ml_dtypes.bfloat16).copy()
    b2p = b2[perm].reshape(1, IH).astype(ml_dtypes.bfloat16)
    Wg16 = np.zeros((H, 16), np.float32)
    Wg16[:, :C] = Wg
    bg16 = np.zeros((P, 16), np.float32)
    bg16[:, :C] = bg

    in_maps = []
    for c in range(NCORES):
        pc = per_core[c]
        xT = np.zeros((IN, NPAD), np.float32)
        xT[:, :NLOC] = x[c * NLOC:(c + 1) * NLOC].T
        in_maps.append({
            "x": x,
            "attr_g": pc["attr_g"],
            "dstf": pc["dstf"],
            "srcix": pc["srcix"],
            "ysrcix": pc["ysrcix"],
            "xT": xT,
            "w1p": w1p,
            "b1p": b1p,
            "W2p": W2p,
            "b2p": b2p,
            "rootw": rootw,
            "bias1r": bias1.reshape(1, H),
            "Wg": Wg16,
            "bg_rep": bg16,
        })

    results = runner.run(in_maps)
    out = np.concatenate(
        [results[c]["out_final"][:NLOC] for c in range(NCORES)], axis=0
    )
    return out.astype(np.float32)


# revision 11
# speedup vs baseline: 1.4367x; 1.2384x over previous
"""GCN+NNConv (edge-MLP message passing) Trainium2 Bass kernel, 8-core SPMD.

Sharding: edges sorted by dst, sharded by dst range (3750 nodes/core).
Each 128-node block's edges are padded to a fixed 6 edge-tiles so all 8
cores run one identical program. Aggregation is done with one-hot merge
matmuls accumulating in PSUM per node block (no scatters). x[src] and
y[src] are fetched with per-tile indirect DMA gathers. One AllGather
shares y across cores for the GCN layer.
"""
import numpy as np

import concourse.bass as bass
import concourse.mybir as mybir
import concourse.tile as tile

# problem constants (hardcoded per contract)
N = 30000
E = 150000
IN = 16
H = 32
C = 10
IH = IN * H          # 512
NCORES = 8
NLOC = N // NCORES   # 3750
NPAD = 3840          # 30 blocks of 128
NB = NPAD // 128     # 30
P = 128

_COMPILED = {}


def _split_multi_waits(nc, max_waits=1):
    """This walrus build allows only one inline sync-wait per instruction;
    hoist extras into single-wait NOPs on the same engine just before."""
    for fn in nc.m.functions:
        for blk in fn.blocks:
            insts = list(blk.instructions)
            new_insts = []
            for inst in insts:
                si = inst.sync_info
                if si is not None and si.on_wait is not None and len(si.on_wait) > max_waits:
                    waits = list(si.on_wait)
                    keep = waits[-max_waits:]
                    extra = waits[:-max_waits]
                    for j, w in enumerate(extra):
                        nop = mybir.InstNoOp(
                            name=f"{inst.name}-waitnop{j}",
                            engine=inst.engine,
                            ins=[], outs=[],
                            sync_info=mybir.SyncInfo(on_wait=[w], on_update=[]),
                        )
                        new_insts.append(nop)
                    inst.sync_info = mybir.SyncInfo(on_wait=keep, on_update=si.on_update)
                new_insts.append(inst)
            blk.instructions = new_insts
    return nc


def _build(tpb):
    """Build the SPMD Bass kernel. tpb = tiles per block (edge capacity/block/128)."""
    T = NB * tpb                 # edge tiles per core
    NG = (T + 3) // 4            # groups of 4 tiles
    f32 = mybir.dt.float32
    bf16 = mybir.dt.bfloat16
    i32 = mybir.dt.int32
    AF = mybir.ActivationFunctionType
    AL = mybir.AluOpType
    AX = mybir.AxisListType

    nc = bass.Bass(num_devices=NCORES)

    # ---- inputs ----
    x_d = nc.dram_tensor("x", [N, IN], f32, kind="ExternalInput")
    GW = (NG + 2) // 3 * 512
    attr_g = nc.dram_tensor("attr_g", [3, GW], bf16, kind="ExternalInput")
    dstf_d = nc.dram_tensor("dstf", [P, T], f32, kind="ExternalInput")
    srcix_d = nc.dram_tensor("srcix", [P, T], i32, kind="ExternalInput")
    ysrcix_d = nc.dram_tensor("ysrcix", [P, T], i32, kind="ExternalInput")
    xT_d = nc.dram_tensor("xT", [IN, NPAD], f32, kind="ExternalInput")
    w1p_d = nc.dram_tensor("w1p", [65, IH], bf16, kind="ExternalInput")
    b1p_d = nc.dram_tensor("b1p", [P, 4], f32, kind="ExternalInput")
    W2p_d = nc.dram_tensor("W2p", [P, 4, IH], bf16, kind="ExternalInput")
    b2p_d = nc.dram_tensor("b2p", [1, IH], bf16, kind="ExternalInput")
    root_d = nc.dram_tensor("rootw", [IN, H], f32, kind="ExternalInput")
    bias1_d = nc.dram_tensor("bias1r", [1, H], f32, kind="ExternalInput")
    Wg_d = nc.dram_tensor("Wg", [H, 16], f32, kind="ExternalInput")
    bg_d = nc.dram_tensor("bg_rep", [P, 16], f32, kind="ExternalInput")

    # ---- outputs ----
    out_d = nc.dram_tensor("out_final", [NPAD, C], f32, kind="ExternalOutput")

    with tile.TileContext(nc) as tc:
        with (
            tc.tile_pool(name="cst", bufs=1) as cst,
            tc.tile_pool(name="big", bufs=1) as big,
            tc.tile_pool(name="wk", bufs=3) as wk,
            tc.tile_pool(name="ht", bufs=2) as htp,
            tc.tile_pool(name="ph", bufs=2, space="PSUM") as ph,
            tc.tile_pool(name="pt", bufs=2, space="PSUM") as pt,
            tc.tile_pool(name="pm", bufs=2, space="PSUM") as pm,
            tc.tile_pool(name="pb", bufs=2, space="PSUM") as pb,
            tc.tile_pool(name="dram", bufs=1, space="DRAM") as dram,
        ):
            # ---- load constants / weights ----
            w1p = cst.tile([65, IH], bf16)
            nc.gpsimd.dma_start(out=w1p[:], in_=w1p_d[:])
            b1p = cst.tile([P, 4], f32)
            nc.gpsimd.dma_start(out=b1p[:], in_=b1p_d[:])
            W2p = cst.tile([P, 4, IH], bf16)
            nc.gpsimd.dma_start(out=W2p[:], in_=W2p_d[:])
            b2p = cst.tile([1, IH], bf16)
            nc.gpsimd.dma_start(out=b2p[:], in_=b2p_d[:])
            rootw = cst.tile([IN, H], f32)
            nc.gpsimd.dma_start(out=rootw[:], in_=root_d[:])
            bias1r = cst.tile([1, H], f32)
            nc.gpsimd.dma_start(out=bias1r[:], in_=bias1_d[:])
            Wg = cst.tile([H, 16], f32)
            nc.gpsimd.dma_start(out=Wg[:], in_=Wg_d[:])
            bg_rep = cst.tile([P, 16], f32)
            nc.gpsimd.dma_start(out=bg_rep[:], in_=bg_d[:])
            ag = cst.tile([65, GW], bf16)
            for k in range(3):
                nc.gpsimd.dma_start(out=ag[k * 32:k * 32 + 1, :], in_=attr_g[k:k + 1, :])
            dstf = cst.tile([P, T], f32)
            nc.gpsimd.dma_start(out=dstf[:], in_=dstf_d[:])
            srcix = cst.tile([P, T], i32)
            nc.gpsimd.dma_start(out=srcix[:], in_=srcix_d[:])
            ysrcix = cst.tile([P, T], i32)
            nc.gpsimd.dma_start(out=ysrcix[:], in_=ysrcix_d[:])
            xTt = cst.tile([IN, NPAD], f32)
            nc.gpsimd.dma_start(out=xTt[:], in_=xT_d[:])

            iota_i = cst.tile([P, P], i32)
            nc.gpsimd.iota(iota_i[:], pattern=[[1, P]], base=0, channel_multiplier=0)
            iotaf = cst.tile([P, P], f32)
            nc.vector.tensor_copy(out=iotaf[:], in_=iota_i[:])
            ones_row = cst.tile([1, P], f32)
            ones_bf = cst.tile([1, P], bf16)
            nc.vector.memset(ones_bf[:], 1.0)
            nc.vector.memset(ones_row[:], 1.0)
            ident = cst.tile([P, P], f32)
            from concourse.masks import make_identity
            make_identity(nc, ident[:])

            # ---- big per-core buffers ----
            xs = big.tile([P, T, IN], f32)          # gathered x[src]
            yg = big.tile([P, T, 16], f32)          # gathered y_full[src]
            summed = big.tile([P, NB, 33], f32)     # phase-A node sums (+count)
            y_own = big.tile([P, NB, 16], f32)
            dinv_a = big.tile([P, NB], f32)

            # ---- phase A-0: gather x[src] for every edge tile ----
            for t in range(T):
                nc.gpsimd.indirect_dma_start(
                    out=xs[:, t, :], out_offset=None, in_=x_d[:],
                    in_offset=bass.IndirectOffsetOnAxis(ap=srcix[:, t:t + 1], axis=0),
                )

            # ---- phase A: per group h, per tile theta/msg/merge ----
            def h_group(u):
                """Compute relu(w1*a+b1) for 4 tiles of group u -> hT [128k, 4kt, 512e]."""
                hT = htp.tile([P, 4, 512], bf16, name="hT", tag="hT")
                bp = (u % 3) * 32
                rhs = ag[bp:bp + 1, (u // 3) * 512:(u // 3) * 512 + 512]
                for kt in range(4):
                    hp = ph.tile([P, 512], f32, name="hp", tag="hp")
                    nc.tensor.matmul(out=hp[:], lhsT=w1p[bp:bp + 1, kt * P:(kt + 1) * P],
                                     rhs=rhs, start=True, stop=True)
                    nc.scalar.activation(out=hT[:, kt, :], in_=hp[:], func=AF.Relu,
                                         bias=b1p[:, kt:kt + 1], scale=1.0)
                return hT

            for b in range(NB):
                mps = pm.tile([P, 33], f32, name="mps", tag="mps")
                for j in range(tpb):
                    t = b * tpb + j
                    r = t % 4
                    if r == 0:
                        hT_cur = h_group(t // 4)
                    # theta for tile t
                    th = pt.tile([P, IH], f32, name="th", tag="th")
                    for kt in range(4):
                        nc.tensor.matmul(out=th[:], lhsT=hT_cur[:, kt, r * P:(r + 1) * P],
                                         rhs=W2p[:, kt, :], start=(kt == 0), stop=False)
                    nc.tensor.matmul(out=th[:], lhsT=ones_bf[:], rhs=b2p[:],
                                     start=False, stop=True)
                    # msg = sum_i xs[:,t,i] * theta[:, (o,i)]
                    prod = wk.tile([P, IH], f32, name="prod", tag="prod")
                    nc.vector.tensor_tensor(
                        out=prod[:],
                        in0=th[:].rearrange("p (o i) -> p o i", i=IN),
                        in1=xs[:, t, None, :].broadcast_to([P, H, IN]),
                        op=AL.mult,
                    )
                    msg = wk.tile([P, 33], f32, name="msg", tag="msg")
                    nc.vector.tensor_reduce(
                        out=msg[:, :H], in_=prod[:].rearrange("p (o i) -> p o i", i=IN),
                        axis=AX.X, op=AL.add,
                    )
                    nc.vector.memset(msg[:, H:H + 1], 1.0)
                    # merge into node-block psum
                    sh = wk.tile([P, 1], f32, name="sh", tag="sh")
                    nc.vector.tensor_scalar_sub(out=sh[:], in0=dstf[:, t:t + 1],
                                                scalar1=float(128 * b))
                    S = wk.tile([P, P], f32, name="S", tag="S")
                    nc.vector.tensor_tensor(out=S[:], in0=sh[:].to_broadcast([P, P]),
                                            in1=iotaf[:], op=AL.is_equal)
                    nc.tensor.matmul(out=mps[:], lhsT=S[:], rhs=msg[:],
                                     start=(j == 0), stop=(j == tpb - 1))
                nc.scalar.copy(out=summed[:, b, :], in_=mps[:])

            # ---- phase B: per node-block ----
            for b in range(NB):
                cnt = summed[:, b, H:H + 1]
                c1 = wk.tile([P, 1], f32, name="c1", tag="c1")
                nc.vector.tensor_scalar_max(out=c1[:], in0=cnt, scalar1=1.0)
                rec = wk.tile([P, 1], f32, name="rec", tag="rec")
                nc.vector.reciprocal(out=rec[:], in_=c1[:])
                aggr = wk.tile([P, H], f32, name="aggr", tag="aggr")
                nc.vector.tensor_scalar_mul(out=aggr[:], in0=summed[:, b, :H], scalar1=rec[:])
                xr = pb.tile([P, H], f32, name="xr", tag="pb")
                nc.tensor.matmul(out=xr[:], lhsT=xTt[:, b * P:(b + 1) * P], rhs=rootw[:],
                                 start=True, stop=False)
                nc.tensor.matmul(out=xr[:], lhsT=ones_row[:], rhs=bias1r[:],
                                 start=False, stop=True)
                pre = wk.tile([P, H], f32, name="pre", tag="pre")
                nc.vector.tensor_tensor(out=pre[:], in0=aggr[:], in1=xr[:], op=AL.add)
                h1 = wk.tile([P, H], f32, name="h1", tag="h1")
                nc.scalar.activation(out=h1[:], in_=pre[:], func=AF.Relu)
                tp = pb.tile([H, P], f32, name="tp", tag="pb")
                nc.tensor.transpose(out=tp[:], in_=h1[:], identity=ident[:])
                h1T = wk.tile([H, P], f32, name="h1T", tag="h1T")
                nc.vector.tensor_copy(out=h1T[:], in_=tp[:])
                xw = pb.tile([P, 16], f32, name="xw", tag="pb")
                nc.tensor.matmul(out=xw[:], lhsT=h1T[:], rhs=Wg[:], start=True, stop=True)
                d1 = wk.tile([P, 1], f32, name="d1", tag="d1")
                nc.vector.tensor_scalar_add(out=d1[:], in0=cnt, scalar1=1.0)
                r2 = wk.tile([P, 1], f32, name="r2", tag="r2")
                nc.vector.reciprocal(out=r2[:], in_=d1[:])
                nc.scalar.sqrt(out=dinv_a[:, b:b + 1], in_=r2[:])
                ysb = wk.tile([P, 16], f32, name="ysb", tag="ysb")
                nc.vector.memset(ysb[:], 0.0)
                nc.vector.tensor_scalar_mul(out=ysb[:, :C], in0=xw[:, :C],
                                            scalar1=dinv_a[:, b:b + 1])
                nc.vector.tensor_copy(out=y_own[:, b, :], in_=ysb[:])

            # ---- AllGather y ----
            ag_in = dram.tile([NPAD, 16], f32)
            y_full = dram.tile([NCORES * NPAD, 16], f32, addr_space="Shared")
            # copy local slice into internal dram bounce then collective
            for b in range(NB):
                nc.gpsimd.dma_start(out=ag_in[b * P:(b + 1) * P, :], in_=y_own[:, b, :])
            nc.gpsimd.collective_compute(
                "AllGather",
                AL.bypass,
                replica_groups=[list(range(NCORES))],
                ins=[ag_in[:].opt()],
                outs=[y_full[:].opt()],
            )

            # ---- phase C-0: gather y_full[src] ----
            for t in range(T):
                nc.gpsimd.indirect_dma_start(
                    out=yg[:, t, :], out_offset=None, in_=y_full[:],
                    in_offset=bass.IndirectOffsetOnAxis(ap=ysrcix[:, t:t + 1], axis=0),
                )

            # ---- phase C: merge + output ----
            for b in range(NB):
                aps = pm.tile([P, 33], f32, name="aps", tag="mps")
                for j in range(tpb):
                    t = b * tpb + j
                    sh = wk.tile([P, 1], f32, name="sh2", tag="sh")
                    nc.vector.tensor_scalar_sub(out=sh[:], in0=dstf[:, t:t + 1],
                                                scalar1=float(128 * b))
                    S = wk.tile([P, P], f32, name="S2", tag="S")
                    nc.vector.tensor_tensor(out=S[:], in0=sh[:].to_broadcast([P, P]),
                                            in1=iotaf[:], op=AL.is_equal)
                    nc.tensor.matmul(out=aps[:, :16], lhsT=S[:], rhs=yg[:, t, :],
                                     start=(j == 0), stop=(j == tpb - 1))
                t3 = wk.tile([P, 16], f32, name="t3", tag="t3")
                nc.vector.tensor_tensor(out=t3[:], in0=aps[:, :16], in1=y_own[:, b, :], op=AL.add)
                t4 = wk.tile([P, 16], f32, name="t4", tag="t4")
                nc.vector.tensor_scalar_mul(out=t4[:], in0=t3[:], scalar1=dinv_a[:, b:b + 1])
                t5 = wk.tile([P, 16], f32, name="t5", tag="t5")
                nc.vector.tensor_tensor(out=t5[:], in0=t4[:], in1=bg_rep[:], op=AL.add)
                mx = wk.tile([P, 1], f32, name="mx", tag="mx")
                nc.vector.tensor_reduce(out=mx[:], in_=t5[:, :C], axis=AX.X, op=AL.max)
                sh2 = wk.tile([P, C], f32, name="shl", tag="shl")
                nc.vector.tensor_scalar_sub(out=sh2[:], in0=t5[:, :C], scalar1=mx[:])
                ex = wk.tile([P, C], f32, name="ex", tag="ex")
                se = wk.tile([P, 1], f32, name="se", tag="se")
                nc.scalar.activation(out=ex[:], in_=sh2[:], func=AF.Exp, accum_out=se[:])
                lse = wk.tile([P, 1], f32, name="lse", tag="lse")
                nc.scalar.activation(out=lse[:], in_=se[:], func=AF.Ln)
                ofin = wk.tile([P, C], f32, name="ofin", tag="ofin")
                nc.vector.tensor_scalar_sub(out=ofin[:], in0=sh2[:], scalar1=lse[:])
                nc.gpsimd.dma_start(out=out_d[b * P:(b + 1) * P, :], in_=ofin[:])

    _split_multi_waits(nc)
    return nc, T


class _Runner:
    """Jit-once PJRT executor for the SPMD Bass kernel (mirrors
    concourse.bass2jax.run_bass_via_pjrt, but reusable across calls)."""

    def __init__(self, nc):
        import jax
        import numpy as _np
        from jax.sharding import Mesh, PartitionSpec
        from jax.experimental.shard_map import shard_map
        from concourse.bass2jax import (
            install_neuronx_cc_hook, _bass_exec_p, partition_id_tensor,
        )

        install_neuronx_cc_hook()
        self.jax = jax
        pname = nc.partition_id_tensor.name if nc.partition_id_tensor else None
        in_names, out_names, out_avals, zero_outs = [], [], [], []
        for alloc in nc.m.functions[0].allocations:
            if not isinstance(alloc, mybir.MemoryLocationSet):
                continue
            name = alloc.memorylocations[0].name
            if alloc.kind == "ExternalInput":
                if name != pname:
                    in_names.append(name)
            elif alloc.kind == "ExternalOutput":
                out_names.append(name)
                shape = tuple(alloc.tensor_shape)
                dtype = mybir.dt.np(alloc.dtype)
                out_avals.append(jax.core.ShapedArray(shape, dtype))
                zero_outs.append(_np.zeros(shape, dtype))
        self.in_names, self.out_names = in_names, out_names
        self.out_avals, self.zero_outs = out_avals, zero_outs
        n_params, n_outs = len(in_names), len(out_avals)
        all_in = in_names + out_names + ([pname] if pname else [])

        def _body(*args):
            operands = list(args)
            if pname is not None:
                operands.append(partition_id_tensor())
            return tuple(_bass_exec_p.bind(
                *operands, out_avals=tuple(out_avals), in_names=tuple(all_in),
                out_names=tuple(out_names), lowering_input_output_aliases=(),
                sim_require_finite=True, sim_require_nnan=True, nc=nc,
            ))

        devices = jax.devices()[:NCORES]
        self.mesh = Mesh(np.asarray(devices), ("core",))
        in_specs = (PartitionSpec("core"),) * (n_params + n_outs)
        out_specs = (PartitionSpec("core"),) * len(out_names)
        self._fn = jax.jit(
            shard_map(_body, mesh=self.mesh, in_specs=in_specs,
                      out_specs=out_specs, check_rep=False),
            donate_argnums=tuple(range(n_params, n_params + n_outs)),
            keep_unused=True,
        )

    def run(self, in_maps):
        import numpy as _np
        concat_in = [
            _np.concatenate([_np.asarray(in_maps[c][n]) for c in range(NCORES)], axis=0)
            for n in self.in_names
        ]
        zeros = [
            _np.zeros((NCORES * z.shape[0], *z.shape[1:]), z.dtype)
            for z in self.zero_outs
        ]
        outs = self._fn(*concat_in, *zeros)
        self.jax.block_until_ready(outs)
        res = []
        for c in range(NCORES):
            d = {}
            for i, name in enumerate(self.out_names):
                a = _np.asarray(outs[i])
                d[name] = a.reshape(NCORES, *self.out_avals[i].shape)[c]
            res.append(d)
        return res


def _get_compiled(tpb):
    if tpb not in _COMPILED:
        nc, T = _build(tpb)
        _COMPILED[tpb] = (_Runner(nc), T)
    return _COMPILED[tpb]


def _prep_core(core, src_s, dst_s, attr_s, tpb):
    """Build per-core padded, block-quantized edge arrays."""
    T = NB * tpb
    EC = T * P
    lo, hi = core * NLOC, (core + 1) * NLOC
    i0, i1 = np.searchsorted(dst_s, lo), np.searchsorted(dst_s, hi)
    src_c = src_s[i0:i1]
    dstl_c = (dst_s[i0:i1] - lo).astype(np.int64)
    attr_c = attr_s[i0:i1]

    src_pad = np.zeros(EC, np.int64)
    dstl_pad = np.full(EC, NPAD - 1, np.int64)
    attr_pad = np.zeros(EC, np.float32)
    blk = dstl_c // P
    cnts = np.bincount(blk, minlength=NB)
    if cnts.max() > tpb * P:
        return None  # caller bumps tpb
    starts = np.searchsorted(blk, np.arange(NB))
    pos = blk * (tpb * P) + (np.arange(len(blk)) - starts[blk])
    src_pad[pos] = src_c
    dstl_pad[pos] = dstl_c
    attr_pad[pos] = attr_c

    owner = src_pad // NLOC
    yrow = owner * NPAD + (src_pad - owner * NLOC)

    srcix = src_pad.reshape(T, P).T.astype(np.int32).copy()
    ysrcix = yrow.reshape(T, P).T.astype(np.int32).copy()
    dstf = dstl_pad.reshape(T, P).T.astype(np.float32).copy()

    # attr groups: group u (4 tiles = 512 edges) -> partition (u%3)*32, cols (u//3)*512
    NG = (T + 3) // 4
    GW = (NG + 2) // 3 * 512
    import ml_dtypes as _mld
    attr_gr = np.zeros((3, GW), _mld.bfloat16)
    ncols = (NG + 2) // 3
    a3 = np.zeros(3 * ncols * 512, np.float32)
    # group u -> row u%3, col-block u//3
    for k in range(3):
        rows = attr_pad.reshape(NG, 512)[k::3]
        attr_gr[k, :rows.size] = rows.reshape(-1).astype(_mld.bfloat16)
    return {"srcix": srcix, "ysrcix": ysrcix, "dstf": dstf, "attr_g": attr_gr}


def kernel(**inputs):
    x = np.asarray(inputs["x"], np.float32)
    ea = np.asarray(inputs["edge_attr"], np.float32).reshape(-1)
    ei = np.asarray(inputs["edge_index"]).astype(np.int64)
    W1 = np.asarray(inputs["W1"], np.float32)
    b1 = np.asarray(inputs["b1"], np.float32)
    W2 = np.asarray(inputs["W2"], np.float32)
    b2 = np.asarray(inputs["b2"], np.float32)
    rootw = np.asarray(inputs["root"], np.float32)
    bias1 = np.asarray(inputs["bias1"], np.float32)
    Wg = np.asarray(inputs["Wg"], np.float32)
    bg = np.asarray(inputs["bg"], np.float32)

    src, dst = ei[0], ei[1]
    order = np.argsort(dst, kind="stable")
    src_s, dst_s, attr_s = src[order], dst[order], ea[order]

    # choose tiles-per-block capacity
    tpb = 6
    per_core = None
    while True:
        per_core = [_prep_core(c, src_s, dst_s, attr_s, tpb) for c in range(NCORES)]
        if all(p is not None for p in per_core):
            break
        tpb += 1

    runner, T = _get_compiled(tpb)

    # weight packing (shared across cores)
    perm = np.arange(IH).reshape(IN, H).T.reshape(-1)   # c'=(o,i) -> orig i*32+o
    import ml_dtypes
    w1p = np.zeros((65, IH), ml_dtypes.bfloat16)
    w1p[[0, 32, 64], :] = W1.reshape(1, IH).astype(ml_dtypes.bfloat16)
    b1p = b1.reshape(4, P).T.astype(np.float32).copy()          # [128, 4]
    import ml_dtypes
    W2p = W2[:, perm].reshape(4, P, IH).transpose(1, 0, 2).astype(ml_dtypes.bfloat16).copy()
    b2p = b2[perm].reshape(1, IH).astype(ml_dtypes.bfloat16)
    Wg16 = np.zeros((H, 16), np.float32)
    Wg16[:, :C] = Wg
    bg16 = np.zeros((P, 16), np.float32)
    bg16[:, :C] = bg

    in_maps = []
    for c in range(NCORES):
        pc = per_core[c]
        xT = np.zeros((IN, NPAD), np.float32)
        xT[:, :NLOC] = x[c * NLOC:(c + 1) * NLOC].T
        in_maps.append({
            "x": x,
            "attr_g": pc["attr_g"],
            "dstf": pc["dstf"],
            "srcix": pc["srcix"],
            "ysrcix": pc["ysrcix"],
            "xT": xT,
            "w1p": w1p,
            "b1p": b1p,
            "W2p": W2p,
            "b2p": b2p,
            "rootw": rootw,
            "bias1r": bias1.reshape(1, H),
            "Wg": Wg16,
            "bg_rep": bg16,
        })

    results = runner.run(in_maps)
    out = np.concatenate(
        [results[c]["out_final"][:NLOC] for c in range(NCORES)], axis=0
    )
    return out.astype(np.float32)


# revision 12
# speedup vs baseline: 1.7856x; 1.2429x over previous
"""GCN+NNConv (edge-MLP message passing) Trainium2 Bass kernel, 8-core SPMD.

Sharding: edges sorted by dst, sharded by dst range (3750 nodes/core).
Each 128-node block's edges are padded to a fixed 6 edge-tiles so all 8
cores run one identical program. Aggregation is done with one-hot merge
matmuls accumulating in PSUM per node block (no scatters). x[src] and
y[src] are fetched with per-tile indirect DMA gathers. One AllGather
shares y across cores for the GCN layer.
"""
import numpy as np

import concourse.bass as bass
import concourse.mybir as mybir
import concourse.tile as tile

# problem constants (hardcoded per contract)
N = 30000
E = 150000
IN = 16
H = 32
C = 10
IH = IN * H          # 512
NCORES = 8
NLOC = N // NCORES   # 3750
NPAD = 3840          # 30 blocks of 128
NB = NPAD // 128     # 30
P = 128

_COMPILED = {}


def _split_multi_waits(nc, max_waits=1):
    """This walrus build allows only one inline sync-wait per instruction;
    hoist extras into single-wait NOPs on the same engine just before."""
    for fn in nc.m.functions:
        for blk in fn.blocks:
            insts = list(blk.instructions)
            new_insts = []
            for inst in insts:
                si = inst.sync_info
                if si is not None and si.on_wait is not None and len(si.on_wait) > max_waits:
                    waits = list(si.on_wait)
                    keep = waits[-max_waits:]
                    extra = waits[:-max_waits]
                    for j, w in enumerate(extra):
                        nop = mybir.InstNoOp(
                            name=f"{inst.name}-waitnop{j}",
                            engine=inst.engine,
                            ins=[], outs=[],
                            sync_info=mybir.SyncInfo(on_wait=[w], on_update=[]),
                        )
                        new_insts.append(nop)
                    inst.sync_info = mybir.SyncInfo(on_wait=keep, on_update=si.on_update)
                new_insts.append(inst)
            blk.instructions = new_insts
    return nc


def _build(tpb):
    """Build the SPMD Bass kernel. tpb = tiles per block (edge capacity/block/128)."""
    T = NB * tpb                 # edge tiles per core
    NG = (T + 3) // 4            # groups of 4 tiles
    f32 = mybir.dt.float32
    bf16 = mybir.dt.bfloat16
    i32 = mybir.dt.int32
    AF = mybir.ActivationFunctionType
    AL = mybir.AluOpType
    AX = mybir.AxisListType

    nc = bass.Bass(num_devices=NCORES)

    # ---- inputs ----
    x_d = nc.dram_tensor("x", [N, IN], bf16, kind="ExternalInput")
    GW = (NG + 2) // 3 * 512
    attr_g = nc.dram_tensor("attr_g", [3, GW], bf16, kind="ExternalInput")
    dstf_d = nc.dram_tensor("dstf", [P, T], f32, kind="ExternalInput")
    srcix_d = nc.dram_tensor("srcix", [P, T], i32, kind="ExternalInput")
    ysrcix_d = nc.dram_tensor("ysrcix", [P, T], i32, kind="ExternalInput")
    xT_d = nc.dram_tensor("xT", [IN, NPAD], f32, kind="ExternalInput")
    w1p_d = nc.dram_tensor("w1p", [65, IH], bf16, kind="ExternalInput")
    b1p_d = nc.dram_tensor("b1p", [P, 4], f32, kind="ExternalInput")
    W2p_d = nc.dram_tensor("W2p", [P, 4, IH], bf16, kind="ExternalInput")
    b2p_d = nc.dram_tensor("b2p", [1, IH], bf16, kind="ExternalInput")
    root_d = nc.dram_tensor("rootw", [IN, H], f32, kind="ExternalInput")
    bias1_d = nc.dram_tensor("bias1r", [1, H], f32, kind="ExternalInput")
    Wg_d = nc.dram_tensor("Wg", [H, 16], f32, kind="ExternalInput")
    bg_d = nc.dram_tensor("bg_rep", [P, 16], f32, kind="ExternalInput")

    # ---- outputs ----
    out_d = nc.dram_tensor("out_final", [NPAD, C], f32, kind="ExternalOutput")

    with tile.TileContext(nc) as tc:
        with (
            tc.tile_pool(name="cst", bufs=1) as cst,
            tc.tile_pool(name="big", bufs=1) as big,
            tc.tile_pool(name="wk", bufs=3) as wk,
            tc.tile_pool(name="ht", bufs=2) as htp,
            tc.tile_pool(name="ph", bufs=2, space="PSUM") as ph,
            tc.tile_pool(name="pt", bufs=2, space="PSUM") as pt,
            tc.tile_pool(name="pm", bufs=2, space="PSUM") as pm,
            tc.tile_pool(name="pb", bufs=2, space="PSUM") as pb,
            tc.tile_pool(name="dram", bufs=1, space="DRAM") as dram,
        ):
            # ---- load constants / weights ----
            w1p = cst.tile([65, IH], bf16)
            nc.gpsimd.dma_start(out=w1p[:], in_=w1p_d[:])
            b1p = cst.tile([P, 4], f32)
            nc.gpsimd.dma_start(out=b1p[:], in_=b1p_d[:])
            W2p = cst.tile([P, 4, IH], bf16)
            nc.gpsimd.dma_start(out=W2p[:], in_=W2p_d[:])
            b2p = cst.tile([1, IH], bf16)
            nc.gpsimd.dma_start(out=b2p[:], in_=b2p_d[:])
            rootw = cst.tile([IN, H], f32)
            nc.gpsimd.dma_start(out=rootw[:], in_=root_d[:])
            bias1r = cst.tile([1, H], f32)
            nc.gpsimd.dma_start(out=bias1r[:], in_=bias1_d[:])
            Wg = cst.tile([H, 16], f32)
            nc.gpsimd.dma_start(out=Wg[:], in_=Wg_d[:])
            bg_rep = cst.tile([P, 16], f32)
            nc.gpsimd.dma_start(out=bg_rep[:], in_=bg_d[:])
            ag = cst.tile([65, GW], bf16)
            for k in range(3):
                nc.gpsimd.dma_start(out=ag[k * 32:k * 32 + 1, :], in_=attr_g[k:k + 1, :])
            dstf = cst.tile([P, T], f32)
            nc.gpsimd.dma_start(out=dstf[:], in_=dstf_d[:])
            srcix = cst.tile([P, T], i32)
            nc.gpsimd.dma_start(out=srcix[:], in_=srcix_d[:])
            ysrcix = cst.tile([P, T], i32)
            nc.gpsimd.dma_start(out=ysrcix[:], in_=ysrcix_d[:])
            xTt = cst.tile([IN, NPAD], f32)
            nc.gpsimd.dma_start(out=xTt[:], in_=xT_d[:])

            iota_i = cst.tile([P, P], i32)
            nc.gpsimd.iota(iota_i[:], pattern=[[1, P]], base=0, channel_multiplier=0)
            iotaf = cst.tile([P, P], f32)
            nc.vector.tensor_copy(out=iotaf[:], in_=iota_i[:])
            ones_row = cst.tile([1, P], f32)
            ones_bf = cst.tile([1, P], bf16)
            nc.vector.memset(ones_bf[:], 1.0)
            nc.vector.memset(ones_row[:], 1.0)
            ident = cst.tile([P, P], f32)
            from concourse.masks import make_identity
            make_identity(nc, ident[:])

            # ---- big per-core buffers ----
            xs = big.tile([P, T, IN], bf16)         # gathered x[src]
            yg = big.tile([P, T, 16], f32)          # gathered y_full[src]
            summed = big.tile([P, NB, 33], f32)     # phase-A node sums (+count)
            y_own = big.tile([P, NB, 16], f32)
            dinv_a = big.tile([P, NB], f32)

            # ---- phase A-0: gather x[src] for every edge tile ----
            for t in range(T):
                nc.gpsimd.indirect_dma_start(
                    out=xs[:, t, :], out_offset=None, in_=x_d[:],
                    in_offset=bass.IndirectOffsetOnAxis(ap=srcix[:, t:t + 1], axis=0),
                )

            # ---- phase A: per group h, per tile theta/msg/merge ----
            def h_group(u):
                """Compute relu(w1*a+b1) for 4 tiles of group u -> hT [128k, 4kt, 512e]."""
                hT = htp.tile([P, 4, 512], bf16, name="hT", tag="hT")
                bp = (u % 3) * 32
                rhs = ag[bp:bp + 1, (u // 3) * 512:(u // 3) * 512 + 512]
                for kt in range(4):
                    hp = ph.tile([P, 512], f32, name="hp", tag="hp")
                    nc.tensor.matmul(out=hp[:], lhsT=w1p[bp:bp + 1, kt * P:(kt + 1) * P],
                                     rhs=rhs, start=True, stop=True)
                    nc.scalar.activation(out=hT[:, kt, :], in_=hp[:], func=AF.Relu,
                                         bias=b1p[:, kt:kt + 1], scale=1.0)
                return hT

            for b in range(NB):
                mps = pm.tile([P, 33], f32, name="mps", tag="mps")
                for j in range(tpb):
                    t = b * tpb + j
                    r = t % 4
                    if r == 0:
                        hT_cur = h_group(t // 4)
                    # theta for tile t
                    th = pt.tile([P, IH], f32, name="th", tag="th")
                    for kt in range(4):
                        nc.tensor.matmul(out=th[:], lhsT=hT_cur[:, kt, r * P:(r + 1) * P],
                                         rhs=W2p[:, kt, :], start=(kt == 0), stop=False)
                    nc.tensor.matmul(out=th[:], lhsT=ones_bf[:], rhs=b2p[:],
                                     start=False, stop=True)
                    # msg = sum_i xs[:,t,i] * theta[:, (o,i)]
                    prod = wk.tile([P, IH], f32, name="prod", tag="prod")
                    nc.vector.tensor_tensor(
                        out=prod[:],
                        in0=th[:].rearrange("p (o i) -> p o i", i=IN),
                        in1=xs[:, t, None, :].broadcast_to([P, H, IN]),
                        op=AL.mult,
                    )
                    msg = wk.tile([P, 33], f32, name="msg", tag="msg")
                    nc.vector.tensor_reduce(
                        out=msg[:, :H], in_=prod[:].rearrange("p (o i) -> p o i", i=IN),
                        axis=AX.X, op=AL.add,
                    )
                    nc.vector.memset(msg[:, H:H + 1], 1.0)
                    # merge into node-block psum
                    sh = wk.tile([P, 1], f32, name="sh", tag="sh")
                    nc.vector.tensor_scalar_sub(out=sh[:], in0=dstf[:, t:t + 1],
                                                scalar1=float(128 * b))
                    S = wk.tile([P, P], f32, name="S", tag="S")
                    nc.vector.tensor_tensor(out=S[:], in0=sh[:].to_broadcast([P, P]),
                                            in1=iotaf[:], op=AL.is_equal)
                    nc.tensor.matmul(out=mps[:], lhsT=S[:], rhs=msg[:],
                                     start=(j == 0), stop=(j == tpb - 1))
                nc.scalar.copy(out=summed[:, b, :], in_=mps[:])

            # ---- phase B: per node-block ----
            for b in range(NB):
                cnt = summed[:, b, H:H + 1]
                c1 = wk.tile([P, 1], f32, name="c1", tag="c1")
                nc.vector.tensor_scalar_max(out=c1[:], in0=cnt, scalar1=1.0)
                rec = wk.tile([P, 1], f32, name="rec", tag="rec")
                nc.vector.reciprocal(out=rec[:], in_=c1[:])
                aggr = wk.tile([P, H], f32, name="aggr", tag="aggr")
                nc.vector.tensor_scalar_mul(out=aggr[:], in0=summed[:, b, :H], scalar1=rec[:])
                xr = pb.tile([P, H], f32, name="xr", tag="pb")
                nc.tensor.matmul(out=xr[:], lhsT=xTt[:, b * P:(b + 1) * P], rhs=rootw[:],
                                 start=True, stop=False)
                nc.tensor.matmul(out=xr[:], lhsT=ones_row[:], rhs=bias1r[:],
                                 start=False, stop=True)
                pre = wk.tile([P, H], f32, name="pre", tag="pre")
                nc.vector.tensor_tensor(out=pre[:], in0=aggr[:], in1=xr[:], op=AL.add)
                h1 = wk.tile([P, H], f32, name="h1", tag="h1")
                nc.scalar.activation(out=h1[:], in_=pre[:], func=AF.Relu)
                tp = pb.tile([H, P], f32, name="tp", tag="pb")
                nc.tensor.transpose(out=tp[:], in_=h1[:], identity=ident[:])
                h1T = wk.tile([H, P], f32, name="h1T", tag="h1T")
                nc.vector.tensor_copy(out=h1T[:], in_=tp[:])
                xw = pb.tile([P, 16], f32, name="xw", tag="pb")
                nc.tensor.matmul(out=xw[:], lhsT=h1T[:], rhs=Wg[:], start=True, stop=True)
                d1 = wk.tile([P, 1], f32, name="d1", tag="d1")
                nc.vector.tensor_scalar_add(out=d1[:], in0=cnt, scalar1=1.0)
                r2 = wk.tile([P, 1], f32, name="r2", tag="r2")
                nc.vector.reciprocal(out=r2[:], in_=d1[:])
                nc.scalar.sqrt(out=dinv_a[:, b:b + 1], in_=r2[:])
                ysb = wk.tile([P, 16], f32, name="ysb", tag="ysb")
                nc.vector.memset(ysb[:], 0.0)
                nc.vector.tensor_scalar_mul(out=ysb[:, :C], in0=xw[:, :C],
                                            scalar1=dinv_a[:, b:b + 1])
                nc.vector.tensor_copy(out=y_own[:, b, :], in_=ysb[:])

            # ---- AllGather y ----
            ag_in = dram.tile([NPAD, 16], f32)
            y_full = dram.tile([NCORES * NPAD, 16], f32, addr_space="Shared")
            # copy local slice into internal dram bounce then collective
            for b in range(NB):
                nc.gpsimd.dma_start(out=ag_in[b * P:(b + 1) * P, :], in_=y_own[:, b, :])
            nc.gpsimd.collective_compute(
                "AllGather",
                AL.bypass,
                replica_groups=[list(range(NCORES))],
                ins=[ag_in[:].opt()],
                outs=[y_full[:].opt()],
            )

            # ---- phase C-0: gather y_full[src] ----
            for t in range(T):
                nc.gpsimd.indirect_dma_start(
                    out=yg[:, t, :], out_offset=None, in_=y_full[:],
                    in_offset=bass.IndirectOffsetOnAxis(ap=ysrcix[:, t:t + 1], axis=0),
                )

            # ---- phase C: merge + output ----
            for b in range(NB):
                aps = pm.tile([P, 33], f32, name="aps", tag="mps")
                for j in range(tpb):
                    t = b * tpb + j
                    sh = wk.tile([P, 1], f32, name="sh2", tag="sh")
                    nc.vector.tensor_scalar_sub(out=sh[:], in0=dstf[:, t:t + 1],
                                                scalar1=float(128 * b))
                    S = wk.tile([P, P], f32, name="S2", tag="S")
                    nc.vector.tensor_tensor(out=S[:], in0=sh[:].to_broadcast([P, P]),
                                            in1=iotaf[:], op=AL.is_equal)
                    nc.tensor.matmul(out=aps[:, :16], lhsT=S[:], rhs=yg[:, t, :],
                                     start=(j == 0), stop=(j == tpb - 1))
                t3 = wk.tile([P, 16], f32, name="t3", tag="t3")
                nc.vector.tensor_tensor(out=t3[:], in0=aps[:, :16], in1=y_own[:, b, :], op=AL.add)
                t4 = wk.tile([P, 16], f32, name="t4", tag="t4")
                nc.vector.tensor_scalar_mul(out=t4[:], in0=t3[:], scalar1=dinv_a[:, b:b + 1])
                t5 = wk.tile([P, 16], f32, name="t5", tag="t5")
                nc.vector.tensor_tensor(out=t5[:], in0=t4[:], in1=bg_rep[:], op=AL.add)
                mx = wk.tile([P, 1], f32, name="mx", tag="mx")
                nc.vector.tensor_reduce(out=mx[:], in_=t5[:, :C], axis=AX.X, op=AL.max)
                sh2 = wk.tile([P, C], f32, name="shl", tag="shl")
                nc.vector.tensor_scalar_sub(out=sh2[:], in0=t5[:, :C], scalar1=mx[:])
                ex = wk.tile([P, C], f32, name="ex", tag="ex")
                se = wk.tile([P, 1], f32, name="se", tag="se")
                nc.scalar.activation(out=ex[:], in_=sh2[:], func=AF.Exp, accum_out=se[:])
                lse = wk.tile([P, 1], f32, name="lse", tag="lse")
                nc.scalar.activation(out=lse[:], in_=se[:], func=AF.Ln)
                ofin = wk.tile([P, C], f32, name="ofin", tag="ofin")
                nc.vector.tensor_scalar_sub(out=ofin[:], in0=sh2[:], scalar1=lse[:])
                nc.gpsimd.dma_start(out=out_d[b * P:(b + 1) * P, :], in_=ofin[:])

    _split_multi_waits(nc)
    return nc, T


class _Runner:
    """Jit-once PJRT executor for the SPMD Bass kernel (mirrors
    concourse.bass2jax.run_bass_via_pjrt, but reusable across calls)."""

    def __init__(self, nc):
        import jax
        import numpy as _np
        from jax.sharding import Mesh, PartitionSpec
        from jax.experimental.shard_map import shard_map
        from concourse.bass2jax import (
            install_neuronx_cc_hook, _bass_exec_p, partition_id_tensor,
        )

        install_neuronx_cc_hook()
        self.jax = jax
        pname = nc.partition_id_tensor.name if nc.partition_id_tensor else None
        in_names, out_names, out_avals, zero_outs = [], [], [], []
        for alloc in nc.m.functions[0].allocations:
            if not isinstance(alloc, mybir.MemoryLocationSet):
                continue
            name = alloc.memorylocations[0].name
            if alloc.kind == "ExternalInput":
                if name != pname:
                    in_names.append(name)
            elif alloc.kind == "ExternalOutput":
                out_names.append(name)
                shape = tuple(alloc.tensor_shape)
                dtype = mybir.dt.np(alloc.dtype)
                out_avals.append(jax.core.ShapedArray(shape, dtype))
                zero_outs.append(_np.zeros(shape, dtype))
        self.in_names, self.out_names = in_names, out_names
        self.out_avals, self.zero_outs = out_avals, zero_outs
        n_params, n_outs = len(in_names), len(out_avals)
        all_in = in_names + out_names + ([pname] if pname else [])

        def _body(*args):
            operands = list(args)
            if pname is not None:
                operands.append(partition_id_tensor())
            return tuple(_bass_exec_p.bind(
                *operands, out_avals=tuple(out_avals), in_names=tuple(all_in),
                out_names=tuple(out_names), lowering_input_output_aliases=(),
                sim_require_finite=True, sim_require_nnan=True, nc=nc,
            ))

        devices = jax.devices()[:NCORES]
        self.mesh = Mesh(np.asarray(devices), ("core",))
        in_specs = (PartitionSpec("core"),) * (n_params + n_outs)
        out_specs = (PartitionSpec("core"),) * len(out_names)
        self._fn = jax.jit(
            shard_map(_body, mesh=self.mesh, in_specs=in_specs,
                      out_specs=out_specs, check_rep=False),
            donate_argnums=tuple(range(n_params, n_params + n_outs)),
            keep_unused=True,
        )

    def run(self, in_maps):
        import numpy as _np
        concat_in = [
            _np.concatenate([_np.asarray(in_maps[c][n]) for c in range(NCORES)], axis=0)
            for n in self.in_names
        ]
        zeros = [
            _np.zeros((NCORES * z.shape[0], *z.shape[1:]), z.dtype)
            for z in self.zero_outs
        ]
        outs = self._fn(*concat_in, *zeros)
        self.jax.block_until_ready(outs)
        res = []
        for c in range(NCORES):
            d = {}
            for i, name in enumerate(self.out_names):
                a = _np.asarray(outs[i])
                d[name] = a.reshape(NCORES, *self.out_avals[i].shape)[c]
            res.append(d)
        return res


def _get_compiled(tpb):
    if tpb not in _COMPILED:
        nc, T = _build(tpb)
        _COMPILED[tpb] = (_Runner(nc), T)
    return _COMPILED[tpb]


def _prep_core(core, src_s, dst_s, attr_s, tpb):
    """Build per-core padded, block-quantized edge arrays."""
    T = NB * tpb
    EC = T * P
    lo, hi = core * NLOC, (core + 1) * NLOC
    i0, i1 = np.searchsorted(dst_s, lo), np.searchsorted(dst_s, hi)
    src_c = src_s[i0:i1]
    dstl_c = (dst_s[i0:i1] - lo).astype(np.int64)
    attr_c = attr_s[i0:i1]

    src_pad = np.zeros(EC, np.int64)
    dstl_pad = np.full(EC, NPAD - 1, np.int64)
    attr_pad = np.zeros(EC, np.float32)
    blk = dstl_c // P
    cnts = np.bincount(blk, minlength=NB)
    if cnts.max() > tpb * P:
        return None  # caller bumps tpb
    starts = np.searchsorted(blk, np.arange(NB))
    pos = blk * (tpb * P) + (np.arange(len(blk)) - starts[blk])
    src_pad[pos] = src_c
    dstl_pad[pos] = dstl_c
    attr_pad[pos] = attr_c

    owner = src_pad // NLOC
    yrow = owner * NPAD + (src_pad - owner * NLOC)

    srcix = src_pad.reshape(T, P).T.astype(np.int32).copy()
    ysrcix = yrow.reshape(T, P).T.astype(np.int32).copy()
    dstf = dstl_pad.reshape(T, P).T.astype(np.float32).copy()

    # attr groups: group u (4 tiles = 512 edges) -> partition (u%3)*32, cols (u//3)*512
    NG = (T + 3) // 4
    GW = (NG + 2) // 3 * 512
    import ml_dtypes as _mld
    attr_gr = np.zeros((3, GW), _mld.bfloat16)
    ncols = (NG + 2) // 3
    a3 = np.zeros(3 * ncols * 512, np.float32)
    # group u -> row u%3, col-block u//3
    for k in range(3):
        rows = attr_pad.reshape(NG, 512)[k::3]
        attr_gr[k, :rows.size] = rows.reshape(-1).astype(_mld.bfloat16)
    return {"srcix": srcix, "ysrcix": ysrcix, "dstf": dstf, "attr_g": attr_gr}


def kernel(**inputs):
    x = np.asarray(inputs["x"], np.float32)
    ea = np.asarray(inputs["edge_attr"], np.float32).reshape(-1)
    ei = np.asarray(inputs["edge_index"]).astype(np.int64)
    W1 = np.asarray(inputs["W1"], np.float32)
    b1 = np.asarray(inputs["b1"], np.float32)
    W2 = np.asarray(inputs["W2"], np.float32)
    b2 = np.asarray(inputs["b2"], np.float32)
    rootw = np.asarray(inputs["root"], np.float32)
    bias1 = np.asarray(inputs["bias1"], np.float32)
    Wg = np.asarray(inputs["Wg"], np.float32)
    bg = np.asarray(inputs["bg"], np.float32)

    src, dst = ei[0], ei[1]
    order = np.argsort(dst, kind="stable")
    src_s, dst_s, attr_s = src[order], dst[order], ea[order]

    # choose tiles-per-block capacity
    tpb = 6
    per_core = None
    while True:
        per_core = [_prep_core(c, src_s, dst_s, attr_s, tpb) for c in range(NCORES)]
        if all(p is not None for p in per_core):
            break
        tpb += 1

    runner, T = _get_compiled(tpb)

    # weight packing (shared across cores)
    perm = np.arange(IH).reshape(IN, H).T.reshape(-1)   # c'=(o,i) -> orig i*32+o
    import ml_dtypes
    w1p = np.zeros((65, IH), ml_dtypes.bfloat16)
    w1p[[0, 32, 64], :] = W1.reshape(1, IH).astype(ml_dtypes.bfloat16)
    b1p = b1.reshape(4, P).T.astype(np.float32).copy()          # [128, 4]
    import ml_dtypes
    W2p = W2[:, perm].reshape(4, P, IH).transpose(1, 0, 2).astype(ml_dtypes.bfloat16).copy()
    b2p = b2[perm].reshape(1, IH).astype(ml_dtypes.bfloat16)
    Wg16 = np.zeros((H, 16), np.float32)
    Wg16[:, :C] = Wg
    bg16 = np.zeros((P, 16), np.float32)
    bg16[:, :C] = bg

    in_maps = []
    for c in range(NCORES):
        pc = per_core[c]
        xT = np.zeros((IN, NPAD), np.float32)
        xT[:, :NLOC] = x[c * NLOC:(c + 1) * NLOC].T
        in_maps.append({
            "x": x.astype(ml_dtypes.bfloat16),
            "attr_g": pc["attr_g"],
            "dstf": pc["dstf"],
            "srcix": pc["srcix"],
            "ysrcix": pc["ysrcix"],
            "xT": xT,
            "w1p": w1p,
            "b1p": b1p,
            "W2p": W2p,
            "b2p": b2p,
            "rootw": rootw,
            "bias1r": bias1.reshape(1, H),
            "Wg": Wg16,
            "bg_rep": bg16,
        })

    results = runner.run(in_maps)
    out = np.concatenate(
        [results[c]["out_final"][:NLOC] for c in range(NCORES)], axis=0
    )
    return out.astype(np.float32)


# revision 13
# speedup vs baseline: 17.4975x; 9.7991x over previous
"""GCN+NNConv (edge-MLP message passing) Trainium2 Bass kernel, 8-core SPMD.

Sharding: edges sorted by dst, sharded by dst range (3750 nodes/core).
Each 128-node block's edges are padded to a fixed 6 edge-tiles so all 8
cores run one identical program. Aggregation is done with one-hot merge
matmuls accumulating in PSUM per node block (no scatters). x[src] and
y[src] are fetched with per-tile indirect DMA gathers. One AllGather
shares y across cores for the GCN layer.
"""
import numpy as np

import concourse.bass as bass
import concourse.mybir as mybir
import concourse.tile as tile

# problem constants (hardcoded per contract)
N = 30000
E = 150000
IN = 16
H = 32
C = 10
IH = IN * H          # 512
NCORES = 8
NLOC = N // NCORES   # 3750
NPAD = 3840          # 30 blocks of 128
NB = NPAD // 128     # 30
P = 128

_COMPILED = {}


def _split_multi_waits(nc, max_waits=1):
    """This walrus build allows only one inline sync-wait per instruction;
    hoist extras into single-wait NOPs on the same engine just before."""
    for fn in nc.m.functions:
        for blk in fn.blocks:
            insts = list(blk.instructions)
            new_insts = []
            for inst in insts:
                si = inst.sync_info
                if si is not None and si.on_wait is not None and len(si.on_wait) > max_waits:
                    waits = list(si.on_wait)
                    keep = waits[-max_waits:]
                    extra = waits[:-max_waits]
                    for j, w in enumerate(extra):
                        nop = mybir.InstNoOp(
                            name=f"{inst.name}-waitnop{j}",
                            engine=inst.engine,
                            ins=[], outs=[],
                            sync_info=mybir.SyncInfo(on_wait=[w], on_update=[]),
                        )
                        new_insts.append(nop)
                    inst.sync_info = mybir.SyncInfo(on_wait=keep, on_update=si.on_update)
                new_insts.append(inst)
            blk.instructions = new_insts
    return nc


def _build(tpb):
    """Build the SPMD Bass kernel. tpb = tiles per block (edge capacity/block/128)."""
    T = NB * tpb                 # edge tiles per core
    NG = (T + 3) // 4            # groups of 4 tiles
    f32 = mybir.dt.float32
    bf16 = mybir.dt.bfloat16
    i32 = mybir.dt.int32
    AF = mybir.ActivationFunctionType
    AL = mybir.AluOpType
    AX = mybir.AxisListType

    nc = bass.Bass(num_devices=NCORES)

    # ---- inputs ----
    x_d = nc.dram_tensor("x", [N, IN], bf16, kind="ExternalInput")
    GW = (NG + 2) // 3 * 512
    attr_g = nc.dram_tensor("attr_g", [3, GW], bf16, kind="ExternalInput")
    dstf_d = nc.dram_tensor("dstf", [P, T], f32, kind="ExternalInput")
    srcix_d = nc.dram_tensor("srcix", [P, T], i32, kind="ExternalInput")
    ysrcix_d = nc.dram_tensor("ysrcix", [P, T], i32, kind="ExternalInput")
    xT_d = nc.dram_tensor("xT", [IN, NPAD], f32, kind="ExternalInput")
    w1p_d = nc.dram_tensor("w1p", [65, IH], bf16, kind="ExternalInput")
    b1p_d = nc.dram_tensor("b1p", [P, 4], f32, kind="ExternalInput")
    W2p_d = nc.dram_tensor("W2p", [P, 4, IH], bf16, kind="ExternalInput")
    b2p_d = nc.dram_tensor("b2p", [1, IH], bf16, kind="ExternalInput")
    root_d = nc.dram_tensor("rootw", [IN, H], f32, kind="ExternalInput")
    bias1_d = nc.dram_tensor("bias1r", [1, H], f32, kind="ExternalInput")
    Wg_d = nc.dram_tensor("Wg", [H, 16], f32, kind="ExternalInput")
    bg_d = nc.dram_tensor("bg_rep", [P, 16], f32, kind="ExternalInput")

    # ---- outputs ----
    out_d = nc.dram_tensor("out_final", [NPAD, C], f32, kind="ExternalOutput")

    with tile.TileContext(nc) as tc:
        with (
            tc.tile_pool(name="cst", bufs=1) as cst,
            tc.tile_pool(name="big", bufs=1) as big,
            tc.tile_pool(name="wk", bufs=3) as wk,
            tc.tile_pool(name="ht", bufs=2) as htp,
            tc.tile_pool(name="ph", bufs=2, space="PSUM") as ph,
            tc.tile_pool(name="pt", bufs=2, space="PSUM") as pt,
            tc.tile_pool(name="pm", bufs=2, space="PSUM") as pm,
            tc.tile_pool(name="pb", bufs=2, space="PSUM") as pb,
            tc.tile_pool(name="dram", bufs=1, space="DRAM") as dram,
        ):
            # ---- load constants / weights ----
            w1p = cst.tile([65, IH], bf16)
            nc.gpsimd.dma_start(out=w1p[:], in_=w1p_d[:])
            b1p = cst.tile([P, 4], f32)
            nc.gpsimd.dma_start(out=b1p[:], in_=b1p_d[:])
            W2p = cst.tile([P, 4, IH], bf16)
            nc.gpsimd.dma_start(out=W2p[:], in_=W2p_d[:])
            b2p = cst.tile([1, IH], bf16)
            nc.gpsimd.dma_start(out=b2p[:], in_=b2p_d[:])
            rootw = cst.tile([IN, H], f32)
            nc.gpsimd.dma_start(out=rootw[:], in_=root_d[:])
            bias1r = cst.tile([1, H], f32)
            nc.gpsimd.dma_start(out=bias1r[:], in_=bias1_d[:])
            Wg = cst.tile([H, 16], f32)
            nc.gpsimd.dma_start(out=Wg[:], in_=Wg_d[:])
            bg_rep = cst.tile([P, 16], f32)
            nc.gpsimd.dma_start(out=bg_rep[:], in_=bg_d[:])
            ag = cst.tile([65, GW], bf16)
            for k in range(3):
                nc.gpsimd.dma_start(out=ag[k * 32:k * 32 + 1, :], in_=attr_g[k:k + 1, :])
            dstf = cst.tile([P, T], f32)
            nc.gpsimd.dma_start(out=dstf[:], in_=dstf_d[:])
            srcix = cst.tile([P, T], i32)
            nc.gpsimd.dma_start(out=srcix[:], in_=srcix_d[:])
            ysrcix = cst.tile([P, T], i32)
            nc.gpsimd.dma_start(out=ysrcix[:], in_=ysrcix_d[:])
            xTt = cst.tile([IN, NPAD], f32)
            nc.gpsimd.dma_start(out=xTt[:], in_=xT_d[:])

            iota_i = cst.tile([P, P], i32)
            nc.gpsimd.iota(iota_i[:], pattern=[[1, P]], base=0, channel_multiplier=0)
            iotaf = cst.tile([P, P], f32)
            nc.vector.tensor_copy(out=iotaf[:], in_=iota_i[:])
            ones_row = cst.tile([1, P], f32)
            ones_bf = cst.tile([1, P], bf16)
            nc.vector.memset(ones_bf[:], 1.0)
            nc.vector.memset(ones_row[:], 1.0)
            ident = cst.tile([P, P], f32)
            from concourse.masks import make_identity
            make_identity(nc, ident[:])

            # ---- big per-core buffers ----
            xs = big.tile([P, T, IN], bf16)         # gathered x[src]
            yg = big.tile([P, T, 16], f32)          # gathered y_full[src]
            summed = big.tile([P, NB, 33], f32)     # phase-A node sums (+count)
            y_own = big.tile([P, NB, 16], f32)
            dinv_a = big.tile([P, NB], f32)

            # ---- phase A-0: gather x[src] for every edge tile ----
            for t in range(T):
                nc.gpsimd.indirect_dma_start(
                    out=xs[:, t, :], out_offset=None, in_=x_d[:],
                    in_offset=bass.IndirectOffsetOnAxis(ap=srcix[:, t:t + 1], axis=0),
                )

            # ---- phase A: per group h, per tile theta/msg/merge ----
            def h_group(u):
                """Compute relu(w1*a+b1) for 4 tiles of group u -> hT [128k, 4kt, 512e]."""
                hT = htp.tile([P, 4, 512], bf16, name="hT", tag="hT")
                bp = (u % 3) * 32
                rhs = ag[bp:bp + 1, (u // 3) * 512:(u // 3) * 512 + 512]
                for kt in range(4):
                    hp = ph.tile([P, 512], f32, name="hp", tag="hp")
                    nc.tensor.matmul(out=hp[:], lhsT=w1p[bp:bp + 1, kt * P:(kt + 1) * P],
                                     rhs=rhs, start=True, stop=True)
                    nc.scalar.activation(out=hT[:, kt, :], in_=hp[:], func=AF.Relu,
                                         bias=b1p[:, kt:kt + 1], scale=1.0)
                return hT

            for b in range(NB):
                mps = pm.tile([P, 33], f32, name="mps", tag="mps")
                for j in range(tpb):
                    t = b * tpb + j
                    r = t % 4
                    if r == 0:
                        hT_cur = h_group(t // 4)
                    # theta for tile t
                    th = pt.tile([P, IH], f32, name="th", tag="th")
                    for kt in range(4):
                        nc.tensor.matmul(out=th[:], lhsT=hT_cur[:, kt, r * P:(r + 1) * P],
                                         rhs=W2p[:, kt, :], start=(kt == 0), stop=False)
                    nc.tensor.matmul(out=th[:], lhsT=ones_bf[:], rhs=b2p[:],
                                     start=False, stop=True)
                    # msg = sum_i xs[:,t,i] * theta[:, (o,i)]
                    prod = wk.tile([P, IH], f32, name="prod", tag="prod")
                    nc.vector.tensor_tensor(
                        out=prod[:],
                        in0=th[:].rearrange("p (o i) -> p o i", i=IN),
                        in1=xs[:, t, None, :].broadcast_to([P, H, IN]),
                        op=AL.mult,
                    )
                    msg = wk.tile([P, 33], f32, name="msg", tag="msg")
                    nc.vector.tensor_reduce(
                        out=msg[:, :H], in_=prod[:].rearrange("p (o i) -> p o i", i=IN),
                        axis=AX.X, op=AL.add,
                    )
                    nc.vector.memset(msg[:, H:H + 1], 1.0)
                    # merge into node-block psum
                    sh = wk.tile([P, 1], f32, name="sh", tag="sh")
                    nc.vector.tensor_scalar_sub(out=sh[:], in0=dstf[:, t:t + 1],
                                                scalar1=float(128 * b))
                    S = wk.tile([P, P], f32, name="S", tag="S")
                    nc.vector.tensor_tensor(out=S[:], in0=sh[:].to_broadcast([P, P]),
                                            in1=iotaf[:], op=AL.is_equal)
                    nc.tensor.matmul(out=mps[:], lhsT=S[:], rhs=msg[:],
                                     start=(j == 0), stop=(j == tpb - 1))
                nc.scalar.copy(out=summed[:, b, :], in_=mps[:])

            # ---- phase B: per node-block ----
            for b in range(NB):
                cnt = summed[:, b, H:H + 1]
                c1 = wk.tile([P, 1], f32, name="c1", tag="c1")
                nc.vector.tensor_scalar_max(out=c1[:], in0=cnt, scalar1=1.0)
                rec = wk.tile([P, 1], f32, name="rec", tag="rec")
                nc.vector.reciprocal(out=rec[:], in_=c1[:])
                aggr = wk.tile([P, H], f32, name="aggr", tag="aggr")
                nc.vector.tensor_scalar_mul(out=aggr[:], in0=summed[:, b, :H], scalar1=rec[:])
                xr = pb.tile([P, H], f32, name="xr", tag="pb")
                nc.tensor.matmul(out=xr[:], lhsT=xTt[:, b * P:(b + 1) * P], rhs=rootw[:],
                                 start=True, stop=False)
                nc.tensor.matmul(out=xr[:], lhsT=ones_row[:], rhs=bias1r[:],
                                 start=False, stop=True)
                pre = wk.tile([P, H], f32, name="pre", tag="pre")
                nc.vector.tensor_tensor(out=pre[:], in0=aggr[:], in1=xr[:], op=AL.add)
                h1 = wk.tile([P, H], f32, name="h1", tag="h1")
                nc.scalar.activation(out=h1[:], in_=pre[:], func=AF.Relu)
                tp = pb.tile([H, P], f32, name="tp", tag="pb")
                nc.tensor.transpose(out=tp[:], in_=h1[:], identity=ident[:])
                h1T = wk.tile([H, P], f32, name="h1T", tag="h1T")
                nc.vector.tensor_copy(out=h1T[:], in_=tp[:])
                xw = pb.tile([P, 16], f32, name="xw", tag="pb")
                nc.tensor.matmul(out=xw[:], lhsT=h1T[:], rhs=Wg[:], start=True, stop=True)
                d1 = wk.tile([P, 1], f32, name="d1", tag="d1")
                nc.vector.tensor_scalar_add(out=d1[:], in0=cnt, scalar1=1.0)
                r2 = wk.tile([P, 1], f32, name="r2", tag="r2")
                nc.vector.reciprocal(out=r2[:], in_=d1[:])
                nc.scalar.sqrt(out=dinv_a[:, b:b + 1], in_=r2[:])
                ysb = wk.tile([P, 16], f32, name="ysb", tag="ysb")
                nc.vector.memset(ysb[:], 0.0)
                nc.vector.tensor_scalar_mul(out=ysb[:, :C], in0=xw[:, :C],
                                            scalar1=dinv_a[:, b:b + 1])
                nc.vector.tensor_copy(out=y_own[:, b, :], in_=ysb[:])

            # ---- AllGather y ----
            ag_in = dram.tile([NPAD, 16], f32)
            y_full = dram.tile([NCORES * NPAD, 16], f32, addr_space="Shared")
            # copy local slice into internal dram bounce then collective
            for b in range(NB):
                nc.gpsimd.dma_start(out=ag_in[b * P:(b + 1) * P, :], in_=y_own[:, b, :])
            nc.gpsimd.collective_compute(
                "AllGather",
                AL.bypass,
                replica_groups=[list(range(NCORES))],
                ins=[ag_in[:].opt()],
                outs=[y_full[:].opt()],
            )

            # ---- phase C-0: gather y_full[src] ----
            for t in range(T):
                nc.gpsimd.indirect_dma_start(
                    out=yg[:, t, :], out_offset=None, in_=y_full[:],
                    in_offset=bass.IndirectOffsetOnAxis(ap=ysrcix[:, t:t + 1], axis=0),
                )

            # ---- phase C: merge + output ----
            for b in range(NB):
                aps = pm.tile([P, 33], f32, name="aps", tag="mps")
                for j in range(tpb):
                    t = b * tpb + j
                    sh = wk.tile([P, 1], f32, name="sh2", tag="sh")
                    nc.vector.tensor_scalar_sub(out=sh[:], in0=dstf[:, t:t + 1],
                                                scalar1=float(128 * b))
                    S = wk.tile([P, P], f32, name="S2", tag="S")
                    nc.vector.tensor_tensor(out=S[:], in0=sh[:].to_broadcast([P, P]),
                                            in1=iotaf[:], op=AL.is_equal)
                    nc.tensor.matmul(out=aps[:, :16], lhsT=S[:], rhs=yg[:, t, :],
                                     start=(j == 0), stop=(j == tpb - 1))
                t3 = wk.tile([P, 16], f32, name="t3", tag="t3")
                nc.vector.tensor_tensor(out=t3[:], in0=aps[:, :16], in1=y_own[:, b, :], op=AL.add)
                t4 = wk.tile([P, 16], f32, name="t4", tag="t4")
                nc.vector.tensor_scalar_mul(out=t4[:], in0=t3[:], scalar1=dinv_a[:, b:b + 1])
                t5 = wk.tile([P, 16], f32, name="t5", tag="t5")
                nc.vector.tensor_tensor(out=t5[:], in0=t4[:], in1=bg_rep[:], op=AL.add)
                mx = wk.tile([P, 1], f32, name="mx", tag="mx")
                nc.vector.tensor_reduce(out=mx[:], in_=t5[:, :C], axis=AX.X, op=AL.max)
                sh2 = wk.tile([P, C], f32, name="shl", tag="shl")
                nc.vector.tensor_scalar_sub(out=sh2[:], in0=t5[:, :C], scalar1=mx[:])
                ex = wk.tile([P, C], f32, name="ex", tag="ex")
                se = wk.tile([P, 1], f32, name="se", tag="se")
                nc.scalar.activation(out=ex[:], in_=sh2[:], func=AF.Exp, accum_out=se[:])
                lse = wk.tile([P, 1], f32, name="lse", tag="lse")
                nc.scalar.activation(out=lse[:], in_=se[:], func=AF.Ln)
                ofin = wk.tile([P, C], f32, name="ofin", tag="ofin")
                nc.vector.tensor_scalar_sub(out=ofin[:], in0=sh2[:], scalar1=lse[:])
                nc.gpsimd.dma_start(out=out_d[b * P:(b + 1) * P, :], in_=ofin[:])

    _split_multi_waits(nc)
    return nc, T


class _Runner:
    """Jit-once PJRT executor for the SPMD Bass kernel (mirrors
    concourse.bass2jax.run_bass_via_pjrt, but reusable across calls)."""

    def __init__(self, nc):
        import jax
        import numpy as _np
        from jax.sharding import Mesh, PartitionSpec
        from jax.experimental.shard_map import shard_map
        from concourse.bass2jax import (
            install_neuronx_cc_hook, _bass_exec_p, partition_id_tensor,
        )

        install_neuronx_cc_hook()
        self.jax = jax
        pname = nc.partition_id_tensor.name if nc.partition_id_tensor else None
        in_names, out_names, out_avals, zero_outs = [], [], [], []
        for alloc in nc.m.functions[0].allocations:
            if not isinstance(alloc, mybir.MemoryLocationSet):
                continue
            name = alloc.memorylocations[0].name
            if alloc.kind == "ExternalInput":
                if name != pname:
                    in_names.append(name)
            elif alloc.kind == "ExternalOutput":
                out_names.append(name)
                shape = tuple(alloc.tensor_shape)
                dtype = mybir.dt.np(alloc.dtype)
                out_avals.append(jax.core.ShapedArray(shape, dtype))
                zero_outs.append(_np.zeros(shape, dtype))
        self.in_names, self.out_names = in_names, out_names
        self.out_avals, self.zero_outs = out_avals, zero_outs
        n_params, n_outs = len(in_names), len(out_avals)
        all_in = in_names + out_names + ([pname] if pname else [])

        def _body(*args):
            operands = list(args)
            if pname is not None:
                operands.append(partition_id_tensor())
            return tuple(_bass_exec_p.bind(
                *operands, out_avals=tuple(out_avals), in_names=tuple(all_in),
                out_names=tuple(out_names), lowering_input_output_aliases=(),
                sim_require_finite=True, sim_require_nnan=True, nc=nc,
            ))

        devices = jax.devices()[:NCORES]
        self.mesh = Mesh(np.asarray(devices), ("core",))
        in_specs = (PartitionSpec("core"),) * (n_params + n_outs)
        out_specs = (PartitionSpec("core"),) * len(out_names)
        self._fn = jax.jit(
            shard_map(_body, mesh=self.mesh, in_specs=in_specs,
                      out_specs=out_specs, check_rep=False),
            donate_argnums=tuple(range(n_params, n_params + n_outs)),
            keep_unused=True,
        )

    def run(self, in_maps):
        import numpy as _np
        import sys as _sys
        concat_in = [
            _np.concatenate([_np.asarray(in_maps[c][n]) for c in range(NCORES)], axis=0)
            for n in self.in_names
        ]
        _sys.modules[__name__]._LAST_CONCAT_IN = concat_in
        zeros = [
            _np.zeros((NCORES * z.shape[0], *z.shape[1:]), z.dtype)
            for z in self.zero_outs
        ]
        outs = self._fn(*concat_in, *zeros)
        self.jax.block_until_ready(outs)
        res = []
        for c in range(NCORES):
            d = {}
            for i, name in enumerate(self.out_names):
                a = _np.asarray(outs[i])
                d[name] = a.reshape(NCORES, *self.out_avals[i].shape)[c]
            res.append(d)
        return res


def _get_compiled(tpb):
    if tpb not in _COMPILED:
        nc, T = _build(tpb)
        _COMPILED[tpb] = (_Runner(nc), T)
    return _COMPILED[tpb]


def _prep_core(core, src_s, dst_s, attr_s, tpb):
    """Build per-core padded, block-quantized edge arrays."""
    T = NB * tpb
    EC = T * P
    lo, hi = core * NLOC, (core + 1) * NLOC
    i0, i1 = np.searchsorted(dst_s, lo), np.searchsorted(dst_s, hi)
    src_c = src_s[i0:i1]
    dstl_c = (dst_s[i0:i1] - lo).astype(np.int64)
    attr_c = attr_s[i0:i1]

    src_pad = np.zeros(EC, np.int64)
    dstl_pad = np.full(EC, NPAD - 1, np.int64)
    attr_pad = np.zeros(EC, np.float32)
    blk = dstl_c // P
    cnts = np.bincount(blk, minlength=NB)
    if cnts.max() > tpb * P:
        return None  # caller bumps tpb
    starts = np.searchsorted(blk, np.arange(NB))
    pos = blk * (tpb * P) + (np.arange(len(blk)) - starts[blk])
    src_pad[pos] = src_c
    dstl_pad[pos] = dstl_c
    attr_pad[pos] = attr_c

    owner = src_pad // NLOC
    yrow = owner * NPAD + (src_pad - owner * NLOC)

    srcix = src_pad.reshape(T, P).T.astype(np.int32).copy()
    ysrcix = yrow.reshape(T, P).T.astype(np.int32).copy()
    dstf = dstl_pad.reshape(T, P).T.astype(np.float32).copy()

    # attr groups: group u (4 tiles = 512 edges) -> partition (u%3)*32, cols (u//3)*512
    NG = (T + 3) // 4
    GW = (NG + 2) // 3 * 512
    import ml_dtypes as _mld
    attr_gr = np.zeros((3, GW), _mld.bfloat16)
    ncols = (NG + 2) // 3
    a3 = np.zeros(3 * ncols * 512, np.float32)
    # group u -> row u%3, col-block u//3
    for k in range(3):
        rows = attr_pad.reshape(NG, 512)[k::3]
        attr_gr[k, :rows.size] = rows.reshape(-1).astype(_mld.bfloat16)
    return {"srcix": srcix, "ysrcix": ysrcix, "dstf": dstf, "attr_g": attr_gr}


def kernel(**inputs):
    x = np.asarray(inputs["x"], np.float32)
    ea = np.asarray(inputs["edge_attr"], np.float32).reshape(-1)
    ei = np.asarray(inputs["edge_index"]).astype(np.int64)
    W1 = np.asarray(inputs["W1"], np.float32)
    b1 = np.asarray(inputs["b1"], np.float32)
    W2 = np.asarray(inputs["W2"], np.float32)
    b2 = np.asarray(inputs["b2"], np.float32)
    rootw = np.asarray(inputs["root"], np.float32)
    bias1 = np.asarray(inputs["bias1"], np.float32)
    Wg = np.asarray(inputs["Wg"], np.float32)
    bg = np.asarray(inputs["bg"], np.float32)

    src, dst = ei[0], ei[1]
    order = np.argsort(dst, kind="stable")
    src_s, dst_s, attr_s = src[order], dst[order], ea[order]

    # choose tiles-per-block capacity
    tpb = 6
    per_core = None
    while True:
        per_core = [_prep_core(c, src_s, dst_s, attr_s, tpb) for c in range(NCORES)]
        if all(p is not None for p in per_core):
            break
        tpb += 1

    runner, T = _get_compiled(tpb)

    # weight packing (shared across cores)
    perm = np.arange(IH).reshape(IN, H).T.reshape(-1)   # c'=(o,i) -> orig i*32+o
    import ml_dtypes
    w1p = np.zeros((65, IH), ml_dtypes.bfloat16)
    w1p[[0, 32, 64], :] = W1.reshape(1, IH).astype(ml_dtypes.bfloat16)
    b1p = b1.reshape(4, P).T.astype(np.float32).copy()          # [128, 4]
    import ml_dtypes
    W2p = W2[:, perm].reshape(4, P, IH).transpose(1, 0, 2).astype(ml_dtypes.bfloat16).copy()
    b2p = b2[perm].reshape(1, IH).astype(ml_dtypes.bfloat16)
    Wg16 = np.zeros((H, 16), np.float32)
    Wg16[:, :C] = Wg
    bg16 = np.zeros((P, 16), np.float32)
    bg16[:, :C] = bg

    in_maps = []
    for c in range(NCORES):
        pc = per_core[c]
        xT = np.zeros((IN, NPAD), np.float32)
        xT[:, :NLOC] = x[c * NLOC:(c + 1) * NLOC].T
        in_maps.append({
            "x": x.astype(ml_dtypes.bfloat16),
            "attr_g": pc["attr_g"],
            "dstf": pc["dstf"],
            "srcix": pc["srcix"],
            "ysrcix": pc["ysrcix"],
            "xT": xT,
            "w1p": w1p,
            "b1p": b1p,
            "W2p": W2p,
            "b2p": b2p,
            "rootw": rootw,
            "bias1r": bias1.reshape(1, H),
            "Wg": Wg16,
            "bg_rep": bg16,
        })

    results = runner.run(in_maps)
    out = np.concatenate(
        [results[c]["out_final"][:NLOC] for c in range(NCORES)], axis=0
    )
    return out.astype(np.float32)
